# revision 77
# baseline (speedup 1.0000x reference)
"""DenseGATConv-style GNN message passing kernel for Trainium2 (Bass/Tile).

Math (per graph b):
    e      = w_edge[edge_attr[b]]            # [N, N] gather from 4-entry table
    adj_w  = adj[b] * e                      # weighted adjacency
    out[b] = adj_w @ x[b] @ W_rel + b_rel + x[b] @ W_root

v4 design (v2 was 45us; v3 traced the walls: ScalarE silu spine ~14.5us,
DVE multiply ~18us, 8 HWDGE semaphore lanes serializing DMA issues):
  * ASSOCIATIVITY: (adj_w @ x) @ W_rel == adj_w @ (x @ W_rel).  The host
    pre-multiplies ys = x @ (lead*W_rel) and rt = x @ W_root + b_rel
    (0.3% of FLOPs); the device aggregation THEN IS the output:
        outT = ysT-contraction over the weighted adjacency, seeded with
        rtT via an identity-lhsT matmul into the same PSUM group.
    No stacked tile, no tail transform, no second x copy.
  * adj ships uint8 (round(adj*255); 1/255 folds into ys), edge_attr
    uint8; decode chain per chunk:
        s = Silu(beta*ea+gamma) [ACT, u8-in]; z = (s+k)*adj_u8 [DVE stt]
    For two middle chunks the multiply runs on the Pool engine instead
    (DVE computes s+k at 4x, Pool does the TT multiply) to balance DVE.
  * DMA: chunks [1,2,2,2,1] per graph alternate the two HWDGE rings;
    issues are emitted in execution order at high priority so the 8
    HWDGE semaphore lanes recycle without false serialization.
  * Output: PSUM [128,512] f32 per graph (both node-halves packed on the
    partition axis via tile_position), one start=True seed, copied to
    f16 and stored transposed; host untransposes.

Sharding: data-parallel over batch B=16 across 8 cores (2 graphs/core);
weights replicated.
"""

import sys
from contextlib import ExitStack

sys.path.insert(0, "/opt/trn_rl_repo")

import numpy as np

_B, _N, _C = 16, 1024, 64
_NCORES = 8
_G = _B // _NCORES  # graphs per core
_P = 128
_NT = _N // _P  # 128-row tiles per graph
_H = 512  # half-graph columns (one PSUM bank of fp32)

# ---- schedule knobs (test.py may override before calling kernel()) ----
TRACE = False
# per-graph DMA chunks: (dma_tiles, [decode slice tile counts]) -- DMA
# granularity (descriptor/line size) decoupled from compute granularity.
# Small first chunk starts the silu spine early; fat rest for queue BW.
CHUNKS = [(1, [1]), (1, [1]), (2, [2]), (2, [2]), (2, [1, 1])]
# ring per (g, ch): alternate, g1 phase-shifted; the two lane-recycled
# issues (emitted last) go to sync so they never block Scalar's silus.
RING = {(0, 0): "sync", (0, 1): "scalar", (0, 2): "sync", (0, 3): "scalar",
        (0, 4): "sync", (1, 0): "scalar", (1, 1): "sync", (1, 2): "scalar",
        (1, 3): "sync", (1, 4): "sync"}
# chunks whose (s+k)*adj multiply runs on Pool (DVE does s+=k at 4x first).
# NOTE: measured HARMFUL -- concurrent Pool TT slows DVE stt 2.3x (SBUF
# contention); keep empty.
POOL_MULT = []
# DMA emission order = HWDGE semaphore lane assignment = execution order.
ISSUE_ORDER = [(0, 0), (0, 1), (0, 2), (0, 3), (0, 4), (1, 0), (1, 1), (1, 2),
               (1, 3), (1, 4)]
# stripe each chunk across both HWDGE rings by partition halves (requires
# <= 4 chunks total so the 8 semaphore lanes aren't recycled by loads)
STRIPE = False
# emit consts/dummy at priority 0 too: Scalar runs table-load+dummy BEFORE
# its DMA issues, so the first silu isn't stuck behind the issue stream
PRIO_CONSTS = True
# tile-pool ring depths: deeper s/z rings let ACT run ahead of DVE and
# absorb DMA arrival jitter
SP_BUFS = 6
AW_BUFS = 8
# chunk 0 packs column-split halves; graph 0 loads them as TWO descriptors
# on BOTH rings in parallel (first data ~1us earlier -> earlier spine start)
SPLIT_C0 = True
LAST_RESULTS = None

_BUILD_CACHE = {}


def _pack_blob(adjq, eaT, xpack, ident):
    """Chunk-major byte blob: per DMA chunk [adj_u8 w | ea_u8 w]; chunk 1
    additionally carries [xpack 2048B] (ys+rtp) and [ident 256B] -- keeps
    total load DMAs at 8 (= HWDGE semaphore lanes, so no recycling)."""
    B = adjq.shape[0]
    id_rows = np.broadcast_to(
        ident.view(np.uint8).reshape(1, _P, 2 * _P), (B, _P, 2 * _P)
    )
    segs = []
    off = 0
    for ch, (ctiles, _slices) in enumerate(CHUNKS):
        w = ctiles * _N
        if ch == 0 and SPLIT_C0:
            # column-split halves: [adjA|eaA|adjB|eaB], 512 cols each
            hw = w // 2
            segs.append(adjq[:, :, off : off + hw])
            segs.append(eaT[:, :, off : off + hw])
            segs.append(adjq[:, :, off + hw : off + w])
            segs.append(eaT[:, :, off + hw : off + w])
        else:
            segs.append(adjq[:, :, off : off + w])
            segs.append(eaT[:, :, off : off + w])
        if ch == 1:
            segs.append(xpack)
            segs.append(id_rows)
        off += w
    return np.ascontiguousarray(np.concatenate(segs, axis=2))


def _poly_coeffs(w_edge):
    w = np.asarray(w_edge, dtype=np.float64).reshape(4)
    V = np.vander(np.arange(4.0), 4, increasing=True)
    return np.linalg.solve(V, w)


def _act_fit(f, w, n_starts=6000, seed=0):
    """Exact 4-point fit w[a] = alpha*f(beta*a+gamma)+delta via random-start
    Gauss-Newton (numpy only).  Returns (beta, gamma, alpha, delta) or None."""
    w = np.asarray(w, dtype=np.float64).reshape(4)
    a4 = np.arange(4.0)
    scale = max(np.max(np.abs(w)), 1e-30)
    rng = np.random.default_rng(seed)
    best = None
    for _ in range(n_starts):
        b = rng.uniform(-4.0, 4.0)
        g = rng.uniform(-8.0, 8.0)
        M = np.stack([f(b * a4 + g), np.ones(4)], axis=1)
        sol, *_ = np.linalg.lstsq(M, w, rcond=None)
        r = M @ sol - w
        v = float(r @ r)
        if best is None or v < best[0]:
            best = (v, b, g, float(sol[0]), float(sol[1]))
    p = np.array(best[1:], dtype=np.float64)
    eps = 1e-6
    for _ in range(200):
        b, g, al, de = p
        r = al * f(b * a4 + g) + de - w
        if np.abs(r).max() < 1e-12 * scale:
            break
        J = np.empty((4, 4))
        for j in range(4):
            q = p.copy()
            q[j] += eps
            J[:, j] = (q[2] * f(q[0] * a4 + q[1]) + q[3] - w - r) / eps
        try:
            step, *_ = np.linalg.lstsq(J, r, rcond=None)
        except np.linalg.LinAlgError:
            return None
        p = p - step
    b, g, al, de = p
    r = al * f(b * a4 + g) + de - w
    if np.abs(r).max() < 1e-9 * scale and abs(al) > 1e-9 * scale:
        return float(b), float(g), float(al), float(de)
    return None


def _fit_chain(w_edge):
    """Pick the device chain for e = w_edge[a], a in {0..3}.

    Preferred: exact silu fit  e = alpha*silu(beta*a+gamma) + delta
    (one ScalarE activation + one DVE stt), then sin (domain checked).
    Falls back to the factored cubic (one ScalarE Square + 2 DVE stt).

    Returns (mode, params, lead): device computes z = chain(a) * adj_u8
    such that true adj_w = (lead/255) * z; lead/255 folds into ys.
    """
    w = np.asarray(w_edge, dtype=np.float64).reshape(4)
    v0, v1, v2, v3 = w
    scale = max(np.max(np.abs(w)), 1e-30)

    def silu(x):
        return x / (1.0 + np.exp(-np.clip(x, -60, 60)))

    fit = _act_fit(silu, w)
    if fit is not None and abs(fit[0]) * 3 + abs(fit[1]) < 30.0:
        b, g, al, de = fit
        return "silu", dict(beta=b, gamma=g, k=float(de / al)), al

    den = (v0 + v2) + 2.0 * v2 - (v1 + v3) - 2.0 * v1
    if abs(den) > 1e-9 * scale:
        d = ((v0 + v2) * v2 - (v1 + v3) * v1) / den
        if abs(v1 - d) > 1e-9 * scale:
            c = (v0 + v2 - 2.0 * d) / (2.0 * (v1 - d))
            if abs(c) < 1.0 - 1e-7:
                b = float(np.arccos(c))
                sb = np.sin(b)
                Pv = v0 - d
                Qv = ((v1 - d) - Pv * c) / sb
                alpha = float(np.hypot(Pv, Qv))
                g = float(np.arctan2(Pv, Qv))
                args = b * np.arange(4.0) + g
                fitv = alpha * np.sin(args) + d
                if (
                    np.abs(fitv - w).max() < 1e-9 * scale
                    and alpha > 1e-9 * scale
                    and np.abs(args).max() <= np.pi
                ):
                    return ("sin", dict(beta=b, gamma=g, k=float(d / alpha)), alpha)

    c0, c1, c2, c3 = _poly_coeffs(w)
    tol = 1e-7 * scale
    if abs(c3) > tol:
        A, Bc, Cc = c2 / c3, c1 / c3, c0 / c3
        roots = np.roots([1.0, A, Bc, Cc])
        r = float(np.real(roots[np.argmin(np.abs(np.imag(roots)))]))
        p = A + r
        q = Bc + p * r
        return "cubic", dict(r=r, h=p / 2.0, v2=q - p * p / 4.0), c3
    if abs(c2) > tol:
        p2, q2 = c1 / c2, c0 / c2
        return "quad", dict(h=p2 / 2.0, v2=q2 - p2 * p2 / 4.0), c2
    if abs(c1) > tol:
        return "linear", dict(r=-c0 / c1), c1
    return "const", dict(), c0


class _Emitter:
    """Holds build state; emits device ops in explicit global order."""

    def __init__(self, nc, pools, dram, mode, params):
        from concourse import mybir

        self.nc = nc
        self.pools = pools
        self.dram = dram
        self.mode = mode
        self.params = params
        self.OP = mybir.AluOpType
        self.AF = mybir.ActivationFunctionType
        self.blob = {}  # (g, ch) -> blob tile
        self.ys = {}
        self.rtp = {}
        self.ident = {}
        self.pk = {}  # g -> packed psum accumulator [128, 512]
        self.chunk_off = np.cumsum([0] + [c[0] for c in CHUNKS[:-1]])
        self.n_tiles = sum(c[0] for c in CHUNKS)

    # ---- DMA issues ------------------------------------------------------
    def issue_blob(self, g, ch):
        nc, pools = self.nc, self.pools
        w = CHUNKS[ch][0] * _N
        cb = 2 * w + (2304 if ch == 1 else 0)
        off = 2 * self.chunk_off[ch] * _N + (2304 if ch > 1 else 0)
        t = pools["blobp"].tile(
            [_P, cb], pools["u8"], name=f"blob{g}_{ch}", tag=f"blob{ch}", bufs=2
        )
        if ch == 0 and SPLIT_C0 and g == 0:
            # column-split halves land via BOTH rings in parallel
            hb = cb // 2
            nc.sync.dma_start(
                out=t[:, 0:hb], in_=self.dram["blob"][g, :, off : off + hb]
            )
            nc.scalar.dma_start(
                out=t[:, hb:cb], in_=self.dram["blob"][g, :, off + hb : off + cb]
            )
        elif STRIPE:
            # partition-halved across BOTH HWDGE rings: 2x arrival speed,
            # queues stay byte-balanced with no ring assignment tuning.
            nc.sync.dma_start(
                out=t[0:64, :], in_=self.dram["blob"][g, 0:64, off : off + cb]
            )
            nc.scalar.dma_start(
                out=t[64:128, :], in_=self.dram["blob"][g, 64:128, off : off + cb]
            )
        else:
            eng = {"sync": nc.sync, "scalar": nc.scalar, "pool": nc.gpsimd}[
                RING[(g, ch)]
            ]
            eng.dma_start(out=t[:], in_=self.dram["blob"][g, :, off : off + cb])
        self.blob[(g, ch)] = t
        if ch == 1:
            self.ys[g] = t[:, 2 * w : 2 * w + 1024].bitcast(pools["bf16"])
            self.rtp[g] = t[:, 2 * w + 1024 : 2 * w + 2048].bitcast(pools["f16"])
            self.ident[g] = t[:, 2 * w + 2048 : 2 * w + 2304].bitcast(pools["f16"])

    # ---- compute ---------------------------------------------------------
    def _decode_slice(self, g, ch, adj_a, ea_a, w, name):
        """silu + (s+k)*adj producing a z tile for one decode slice."""
        nc, pools, OP, AF = self.nc, self.pools, self.OP, self.AF
        z = pools["awp"].tile([_P, w], pools["bf16"], name=f"z{name}", tag="z")
        if self.mode in ("silu", "sin"):
            s_t = pools["sp"].tile([_P, w], pools["f16"], name=f"s{name}", tag="s")
            nc.scalar.activation(
                s_t[:], ea_a,
                AF.Silu if self.mode == "silu" else AF.Sin,
                bias=pools["abias_sb"][:, 0:1],
                scale=float(self.params["beta"]),
            )
            if (g, ch) in POOL_MULT:
                nc.vector.tensor_scalar(
                    s_t[:], s_t[:], float(self.params["k"]), None, OP.add
                )
                nc.gpsimd.tensor_tensor(z[:], s_t[:], adj_a, OP.mult)
            else:
                nc.vector.scalar_tensor_tensor(
                    z[:], s_t[:], float(self.params["k"]), adj_a, OP.add, OP.mult
                )
        elif self.mode in ("cubic", "quad"):
            s_t = pools["sp"].tile([_P, w], pools["f16"], name=f"s{name}", tag="s")
            nc.scalar.activation(
                s_t[:], ea_a, AF.Square,
                bias=pools["abias_sb"][:, 0:1], scale=1.0,
            )
            if self.mode == "cubic":
                qt = pools["qtp"].tile([_P, w], pools["f16"], name=f"q{name}", tag="q")
                nc.vector.scalar_tensor_tensor(
                    qt[:], ea_a, float(self.params["r"]), adj_a,
                    OP.subtract, OP.mult,
                )
                nc.vector.scalar_tensor_tensor(
                    z[:], s_t[:], float(self.params["v2"]), qt[:],
                    OP.add, OP.mult,
                )
            else:
                nc.vector.scalar_tensor_tensor(
                    z[:], s_t[:], float(self.params["v2"]), adj_a,
                    OP.add, OP.mult,
                )
        elif self.mode == "linear":
            nc.vector.scalar_tensor_tensor(
                z[:], ea_a, float(self.params["r"]), adj_a,
                OP.subtract, OP.mult,
            )
        else:  # const
            nc.vector.tensor_copy(z[:], adj_a)
        return z

    def seed(self, g):
        """Open the graph's single PSUM group with outT = I.T @ rtp."""
        self.pk[g] = self.pools["ps"].tile(
            [_P, _H], self.pools["f32"], tag=f"pk{g}", name=f"pk{g}"
        )
        self.nc.tensor.matmul(
            self.pk[g][:],
            lhsT=self.ident[g][:, :],
            rhs=self.rtp[g][:, :],
            start=True, stop=False,
            skip_group_check=True,
        )

    def chunk_ops(self, g, ch):
        """Decode slices of DMA chunk ch, each followed by its agg matmuls."""
        nc = self.nc
        wT = CHUNKS[ch][0] * _N
        blob_t = self.blob[(g, ch)]
        if ch == 0 and SPLIT_C0:
            # col-split layout [adjA|eaA|adjB|eaB]: slice h feeds node-half h
            for h in range(2):
                adj_a = blob_t[:, h * _N : h * _N + _H]
                ea_a = blob_t[:, h * _N + _H : (h + 1) * _N]
                z = self._decode_slice(g, ch, adj_a, ea_a, _H, f"{g}_{ch}_{h}")
                nc.tensor.matmul(
                    self.pk[g][h * _C : (h + 1) * _C, :],
                    lhsT=self.ys[g][:, 0:_C],
                    rhs=z[:, :],
                    start=False, stop=False,
                    skip_group_check=True,
                )
            return
        s_off = 0  # tile offset within the DMA chunk
        for si, stiles in enumerate(CHUNKS[ch][1]):
            w = stiles * _N
            adj_a = blob_t[:, s_off * _N : s_off * _N + w]
            ea_a = blob_t[:, wT + s_off * _N : wT + s_off * _N + w]
            z = self._decode_slice(g, ch, adj_a, ea_a, w, f"{g}_{ch}_{si}")
            for sub in range(stiles):
                jt = self.chunk_off[ch] + s_off + sub
                last = jt == self.n_tiles - 1
                for half in range(2):
                    nc.tensor.matmul(
                        self.pk[g][half * _C : (half + 1) * _C, :],
                        lhsT=self.ys[g][:, jt * _C : (jt + 1) * _C],
                        rhs=z[:, sub * _N + half * _H : sub * _N + (half + 1) * _H],
                        start=False, stop=(last and half == 1),
                        skip_group_check=True,
                    )
            s_off += stiles

    def tail_out(self, g):
        """PSUM -> f16 copy split by partition halves (each waits only its
        own half's last matmul; vector+scalar run concurrently), then two
        stores on the sync ring."""
        outb = self.pools["outp"].tile([_P, _H], self.pools["f16"], tag=f"outb{g}")
        self.nc.vector.tensor_copy(outb[0:_C, :], self.pk[g][0:_C, :])
        self.nc.scalar.copy(out=outb[_C:_P, :], in_=self.pk[g][_C:_P, :])
        self.nc.sync.dma_start(out=self.dram["out"][g, 0:_C, :], in_=outb[0:_C, :])
        self.nc.sync.dma_start(out=self.dram["out"][g, _C:_P, :], in_=outb[_C:_P, :])


def _build_module(mode, params):
    import concourse.bass as bass  # noqa: F401
    from concourse import bacc, mybir
    from concourse.tile import TileContext

    f32 = mybir.dt.float32
    f16 = mybir.dt.float16
    bf16 = mybir.dt.bfloat16
    u8 = mybir.dt.uint8

    nc = bacc.Bacc(
        "TRN2", target_bir_lowering=False, debug=False, num_devices=_NCORES
    )

    blob_w = _NT * _N * 2 + 2304
    dram = {
        "blob": nc.dram_tensor("blob", [_G, _P, blob_w], u8, kind="ExternalInput"),
        "out": nc.dram_tensor("out", [_G, _P, _H], f16, kind="ExternalOutput"),
    }

    pool_specs = [
        ("consts", 1, None),
        ("blobp", 1, None),
        ("sp", SP_BUFS, None),
        ("awp", AW_BUFS, None),
        ("outp", 1, None),
        ("ps", 1, "PSUM"),
    ]
    if mode == "cubic":
        pool_specs.insert(3, ("qtp", 2, None))

    with TileContext(nc) as tc, ExitStack() as ctx:
        pools = {"f32": f32, "f16": f16, "bf16": bf16, "u8": u8, "tc": tc}
        for name, bufs, space in pool_specs:
            kw = {"space": space} if space else {}
            pools[name] = ctx.enter_context(tc.tile_pool(name=name, bufs=bufs, **kw))

        prio = tc.high_priority() if PRIO_CONSTS else None
        if prio:
            prio.__enter__()
        if mode in ("silu", "sin", "cubic", "quad"):
            ab = pools["consts"].tile([_P, 1], f32, tag="abias")
            bias_val = params["gamma"] if mode in ("silu", "sin") else params["h"]
            nc.vector.memset(ab[:], float(bias_val))
            pools["abias_sb"] = ab

        if mode in ("silu", "sin"):
            # dummy activation exactly mirroring the real one (u8 input,
            # f16 out, AP bias + float scale) as the FIRST Scalar
            # instruction: pulls the ACT_TABLE_LOAD into startup dead-time.
            dmu = pools["consts"].tile([_P, 1], u8, tag="dummy_in")
            nc.gpsimd.memset(dmu[:], 0)
            dout = pools["consts"].tile([_P, 1], f16, tag="dummy_out")
            af = (
                mybir.ActivationFunctionType.Silu
                if mode == "silu"
                else mybir.ActivationFunctionType.Sin
            )
            nc.scalar.activation(
                dout[:], dmu[:], af, bias=ab[:, 0:1],
                scale=float(params["beta"]),
            )
        if prio:
            prio.__exit__(None, None, None)

        em = _Emitter(nc, pools, dram, mode, params)

        # ---- all input DMA issues in execution order at priority 0 so the
        # 8 HWDGE semaphore lanes recycle without false serialization.
        with tc.high_priority():
            for item in ISSUE_ORDER:
                em.issue_blob(*item)

        for g in range(_G):
            em.seed(g)
            for ch in range(len(CHUNKS)):
                em.chunk_ops(g, ch)
            em.tail_out(g)

    nc.finalize()
    return nc


def _get_module(w_edge):
    mode, params, lead = _fit_chain(w_edge)
    key = (
        mode,
        tuple(sorted((k, round(v, 15)) for k, v in params.items())),
        tuple((t, tuple(s)) for t, s in CHUNKS),
        tuple(sorted(RING.items())),
        tuple(POOL_MULT),
        tuple(map(str, ISSUE_ORDER)),
        STRIPE,
        PRIO_CONSTS,
        SP_BUFS,
        AW_BUFS,
        SPLIT_C0,
    )
    if key not in _BUILD_CACHE:
        _BUILD_CACHE[key] = _build_module(mode, params)
    return _BUILD_CACHE[key], lead


def kernel(x, adj, edge_attr, W_rel, b_rel, W_root, w_edge):
    global LAST_RESULTS
    from concourse import mybir
    from concourse.bass_utils import run_bass_kernel_spmd

    bf16np = mybir.dt.np(mybir.dt.bfloat16)
    f16np = mybir.dt.np(mybir.dt.float16)

    x = np.asarray(x, dtype=np.float32)
    adj = np.asarray(adj, dtype=np.float32)
    ea = np.asarray(edge_attr, dtype=np.int32).reshape(_B, _N, _N)
    W_rel = np.asarray(W_rel, dtype=np.float64)
    W_root = np.asarray(W_root, dtype=np.float64)
    b_rel = np.asarray(b_rel, dtype=np.float64).reshape(1, _C)
    w_edge = np.asarray(w_edge)

    nc, lead = _get_module(w_edge)

    def tile_rows(a):
        """[B, N, F] -> [B, 128, NT*F]: row j*128+p of graph b lands at
        [b, p, j*F:(j+1)*F] -- one contiguous free-dim line per partition."""
        B, N, F = a.shape
        return np.ascontiguousarray(
            a.reshape(B, _NT, _P, F).transpose(0, 2, 1, 3).reshape(B, _P, _NT * F)
        )

    # transposed + row-tiled layouts; adj quantized to u8 (x255)
    adjq = np.rint(
        tile_rows(np.ascontiguousarray(adj.transpose(0, 2, 1))) * 255.0
    ).astype(np.uint8)
    eaT = tile_rows(np.ascontiguousarray(ea.transpose(0, 2, 1))).astype(np.uint8)

    # associativity folds: ys = x @ (lead/255 * W_rel); rt = x @ W_root + b_rel
    x64 = x.astype(np.float64)
    ys = tile_rows((x64 @ (lead / 255.0 * W_rel)).astype(np.float32)).astype(bf16np)
    rt = (x64 @ W_root + b_rel).astype(np.float32)  # [B, N, C]
    # rtp[g, h*64+c, i] = rt[g, h*512+i, c]
    rtp = np.ascontiguousarray(
        rt.reshape(_B, 2, _H, _C).transpose(0, 1, 3, 2).reshape(_B, _P, _H)
    ).astype(f16np)
    xpack = np.ascontiguousarray(
        np.concatenate([ys.view(np.uint8), rtp.view(np.uint8)], axis=2)
    )
    ident = np.eye(_P, dtype=f16np)
    blob = _pack_blob(adjq, eaT, xpack, ident)

    in_maps = []
    for c in range(_NCORES):
        sl = slice(c * _G, (c + 1) * _G)
        in_maps.append({"blob": blob[sl]})

    res = run_bass_kernel_spmd(nc, in_maps, list(range(_NCORES)), trace=TRACE)
    LAST_RESULTS = res
    # out staged [G, 128, 512] f16: out[g, h*512+i, c] = outs[g, h*64+c, i]
    outs = np.concatenate(
        [np.asarray(res.results[c]["out"]) for c in range(_NCORES)], axis=0
    ).astype(np.float32)
    out = (
        outs.reshape(_B, 2, _C, _H)
        .transpose(0, 1, 3, 2)
        .reshape(_B, _N, _C)
    )
    return np.ascontiguousarray(out)


# revision 78
# speedup vs baseline: 1.0007x; 1.0007x over previous
"""DenseGATConv-style GNN message passing kernel for Trainium2 (Bass/Tile).

Math (per graph b):
    e      = w_edge[edge_attr[b]]            # [N, N] gather from 4-entry table
    adj_w  = adj[b] * e                      # weighted adjacency
    out[b] = adj_w @ x[b] @ W_rel + b_rel + x[b] @ W_root

v4 design (v2 was 45us; v3 traced the walls: ScalarE silu spine ~14.5us,
DVE multiply ~18us, 8 HWDGE semaphore lanes serializing DMA issues):
  * ASSOCIATIVITY: (adj_w @ x) @ W_rel == adj_w @ (x @ W_rel).  The host
    pre-multiplies ys = x @ (lead*W_rel) and rt = x @ W_root + b_rel
    (0.3% of FLOPs); the device aggregation THEN IS the output:
        outT = ysT-contraction over the weighted adjacency, seeded with
        rtT via an identity-lhsT matmul into the same PSUM group.
    No stacked tile, no tail transform, no second x copy.
  * adj ships uint8 (round(adj*255); 1/255 folds into ys), edge_attr
    uint8; decode chain per chunk:
        s = Silu(beta*ea+gamma) [ACT, u8-in]; z = (s+k)*adj_u8 [DVE stt]
    For two middle chunks the multiply runs on the Pool engine instead
    (DVE computes s+k at 4x, Pool does the TT multiply) to balance DVE.
  * DMA: chunks [1,2,2,2,1] per graph alternate the two HWDGE rings;
    issues are emitted in execution order at high priority so the 8
    HWDGE semaphore lanes recycle without false serialization.
  * Output: PSUM [128,512] f32 per graph (both node-halves packed on the
    partition axis via tile_position), one start=True seed, copied to
    f16 and stored transposed; host untransposes.

Sharding: data-parallel over batch B=16 across 8 cores (2 graphs/core);
weights replicated.
"""

import sys
from contextlib import ExitStack

sys.path.insert(0, "/opt/trn_rl_repo")

import numpy as np

_B, _N, _C = 16, 1024, 64
_NCORES = 8
_G = _B // _NCORES  # graphs per core
_P = 128
_NT = _N // _P  # 128-row tiles per graph
_H = 512  # half-graph columns (one PSUM bank of fp32)

# ---- schedule knobs (test.py may override before calling kernel()) ----
TRACE = False
# per-graph DMA chunks: (dma_tiles, [decode slice tile counts]) -- DMA
# granularity (descriptor/line size) decoupled from compute granularity.
# Small first chunk starts the silu spine early; fat rest for queue BW.
CHUNKS = [(1, [1]), (1, [1]), (2, [2]), (2, [2]), (2, [1, 1])]
# ring per (g, ch): alternate, g1 phase-shifted; the two lane-recycled
# issues (emitted last) go to sync so they never block Scalar's silus.
RING = {(0, 0): "sync", (0, 1): "scalar", (0, 2): "sync", (0, 3): "scalar",
        (0, 4): "sync", (1, 0): "scalar", (1, 1): "sync", (1, 2): "scalar",
        (1, 3): "sync", (1, 4): "sync"}
# chunks whose (s+k)*adj multiply runs on Pool (DVE does s+=k at 4x first).
# NOTE: measured HARMFUL -- concurrent Pool TT slows DVE stt 2.3x (SBUF
# contention); keep empty.
POOL_MULT = []
# DMA emission order = HWDGE semaphore lane assignment = execution order.
ISSUE_ORDER = [(0, 0), (0, 1), (0, 2), (0, 3), (0, 4), (1, 0), (1, 1), (1, 2),
               (1, 3), (1, 4)]
# stripe each chunk across both HWDGE rings by partition halves (requires
# <= 4 chunks total so the 8 semaphore lanes aren't recycled by loads)
STRIPE = False
# emit consts/dummy at priority 0 too: Scalar runs table-load+dummy BEFORE
# its DMA issues, so the first silu isn't stuck behind the issue stream
PRIO_CONSTS = True
# tile-pool ring depths: deeper s/z rings let ACT run ahead of DVE and
# absorb DMA arrival jitter
SP_BUFS = 6
AW_BUFS = 8
# chunk 0 packs column-split halves; graph 0 loads them as TWO descriptors
# on BOTH rings in parallel (first data ~1us earlier -> earlier spine start)
SPLIT_C0 = True
# which DMA chunk carries the xpack/ident payload (PE-only consumers can
# tolerate late arrival; the decode spine cannot)
XCARRIER = 2
LAST_RESULTS = None

_BUILD_CACHE = {}


def _pack_blob(adjq, eaT, xpack, ident):
    """Chunk-major byte blob: per DMA chunk [adj_u8 w | ea_u8 w]; chunk 1
    additionally carries [xpack 2048B] (ys+rtp) and [ident 256B] -- keeps
    total load DMAs at 8 (= HWDGE semaphore lanes, so no recycling)."""
    B = adjq.shape[0]
    id_rows = np.broadcast_to(
        ident.view(np.uint8).reshape(1, _P, 2 * _P), (B, _P, 2 * _P)
    )
    segs = []
    off = 0
    for ch, (ctiles, _slices) in enumerate(CHUNKS):
        w = ctiles * _N
        if ch == 0 and SPLIT_C0:
            # column-split halves: [adjA|eaA|adjB|eaB], 512 cols each
            hw = w // 2
            segs.append(adjq[:, :, off : off + hw])
            segs.append(eaT[:, :, off : off + hw])
            segs.append(adjq[:, :, off + hw : off + w])
            segs.append(eaT[:, :, off + hw : off + w])
        else:
            segs.append(adjq[:, :, off : off + w])
            segs.append(eaT[:, :, off : off + w])
        if ch == XCARRIER:
            segs.append(xpack)
            segs.append(id_rows)
        off += w
    return np.ascontiguousarray(np.concatenate(segs, axis=2))


def _poly_coeffs(w_edge):
    w = np.asarray(w_edge, dtype=np.float64).reshape(4)
    V = np.vander(np.arange(4.0), 4, increasing=True)
    return np.linalg.solve(V, w)


def _act_fit(f, w, n_starts=6000, seed=0):
    """Exact 4-point fit w[a] = alpha*f(beta*a+gamma)+delta via random-start
    Gauss-Newton (numpy only).  Returns (beta, gamma, alpha, delta) or None."""
    w = np.asarray(w, dtype=np.float64).reshape(4)
    a4 = np.arange(4.0)
    scale = max(np.max(np.abs(w)), 1e-30)
    rng = np.random.default_rng(seed)
    best = None
    for _ in range(n_starts):
        b = rng.uniform(-4.0, 4.0)
        g = rng.uniform(-8.0, 8.0)
        M = np.stack([f(b * a4 + g), np.ones(4)], axis=1)
        sol, *_ = np.linalg.lstsq(M, w, rcond=None)
        r = M @ sol - w
        v = float(r @ r)
        if best is None or v < best[0]:
            best = (v, b, g, float(sol[0]), float(sol[1]))
    p = np.array(best[1:], dtype=np.float64)
    eps = 1e-6
    for _ in range(200):
        b, g, al, de = p
        r = al * f(b * a4 + g) + de - w
        if np.abs(r).max() < 1e-12 * scale:
            break
        J = np.empty((4, 4))
        for j in range(4):
            q = p.copy()
            q[j] += eps
            J[:, j] = (q[2] * f(q[0] * a4 + q[1]) + q[3] - w - r) / eps
        try:
            step, *_ = np.linalg.lstsq(J, r, rcond=None)
        except np.linalg.LinAlgError:
            return None
        p = p - step
    b, g, al, de = p
    r = al * f(b * a4 + g) + de - w
    if np.abs(r).max() < 1e-9 * scale and abs(al) > 1e-9 * scale:
        return float(b), float(g), float(al), float(de)
    return None


def _fit_chain(w_edge):
    """Pick the device chain for e = w_edge[a], a in {0..3}.

    Preferred: exact silu fit  e = alpha*silu(beta*a+gamma) + delta
    (one ScalarE activation + one DVE stt), then sin (domain checked).
    Falls back to the factored cubic (one ScalarE Square + 2 DVE stt).

    Returns (mode, params, lead): device computes z = chain(a) * adj_u8
    such that true adj_w = (lead/255) * z; lead/255 folds into ys.
    """
    w = np.asarray(w_edge, dtype=np.float64).reshape(4)
    v0, v1, v2, v3 = w
    scale = max(np.max(np.abs(w)), 1e-30)

    def silu(x):
        return x / (1.0 + np.exp(-np.clip(x, -60, 60)))

    fit = _act_fit(silu, w)
    if fit is not None and abs(fit[0]) * 3 + abs(fit[1]) < 30.0:
        b, g, al, de = fit
        return "silu", dict(beta=b, gamma=g, k=float(de / al)), al

    den = (v0 + v2) + 2.0 * v2 - (v1 + v3) - 2.0 * v1
    if abs(den) > 1e-9 * scale:
        d = ((v0 + v2) * v2 - (v1 + v3) * v1) / den
        if abs(v1 - d) > 1e-9 * scale:
            c = (v0 + v2 - 2.0 * d) / (2.0 * (v1 - d))
            if abs(c) < 1.0 - 1e-7:
                b = float(np.arccos(c))
                sb = np.sin(b)
                Pv = v0 - d
                Qv = ((v1 - d) - Pv * c) / sb
                alpha = float(np.hypot(Pv, Qv))
                g = float(np.arctan2(Pv, Qv))
                args = b * np.arange(4.0) + g
                fitv = alpha * np.sin(args) + d
                if (
                    np.abs(fitv - w).max() < 1e-9 * scale
                    and alpha > 1e-9 * scale
                    and np.abs(args).max() <= np.pi
                ):
                    return ("sin", dict(beta=b, gamma=g, k=float(d / alpha)), alpha)

    c0, c1, c2, c3 = _poly_coeffs(w)
    tol = 1e-7 * scale
    if abs(c3) > tol:
        A, Bc, Cc = c2 / c3, c1 / c3, c0 / c3
        roots = np.roots([1.0, A, Bc, Cc])
        r = float(np.real(roots[np.argmin(np.abs(np.imag(roots)))]))
        p = A + r
        q = Bc + p * r
        return "cubic", dict(r=r, h=p / 2.0, v2=q - p * p / 4.0), c3
    if abs(c2) > tol:
        p2, q2 = c1 / c2, c0 / c2
        return "quad", dict(h=p2 / 2.0, v2=q2 - p2 * p2 / 4.0), c2
    if abs(c1) > tol:
        return "linear", dict(r=-c0 / c1), c1
    return "const", dict(), c0


class _Emitter:
    """Holds build state; emits device ops in explicit global order."""

    def __init__(self, nc, pools, dram, mode, params):
        from concourse import mybir

        self.nc = nc
        self.pools = pools
        self.dram = dram
        self.mode = mode
        self.params = params
        self.OP = mybir.AluOpType
        self.AF = mybir.ActivationFunctionType
        self.blob = {}  # (g, ch) -> blob tile
        self.ys = {}
        self.rtp = {}
        self.ident = {}
        self.pk = {}  # g -> packed psum accumulator [128, 512]
        self.chunk_off = np.cumsum([0] + [c[0] for c in CHUNKS[:-1]])
        self.n_tiles = sum(c[0] for c in CHUNKS)

    # ---- DMA issues ------------------------------------------------------
    def issue_blob(self, g, ch):
        nc, pools = self.nc, self.pools
        w = CHUNKS[ch][0] * _N
        cb = 2 * w + (2304 if ch == XCARRIER else 0)
        off = 2 * self.chunk_off[ch] * _N + (2304 if ch > XCARRIER else 0)
        t = pools["blobp"].tile(
            [_P, cb], pools["u8"], name=f"blob{g}_{ch}", tag=f"blob{ch}", bufs=2
        )
        if ch == 0 and SPLIT_C0 and g == 0:
            # column-split halves land via BOTH rings in parallel
            hb = cb // 2
            nc.sync.dma_start(
                out=t[:, 0:hb], in_=self.dram["blob"][g, :, off : off + hb]
            )
            nc.scalar.dma_start(
                out=t[:, hb:cb], in_=self.dram["blob"][g, :, off + hb : off + cb]
            )
        elif STRIPE:
            # partition-halved across BOTH HWDGE rings: 2x arrival speed,
            # queues stay byte-balanced with no ring assignment tuning.
            nc.sync.dma_start(
                out=t[0:64, :], in_=self.dram["blob"][g, 0:64, off : off + cb]
            )
            nc.scalar.dma_start(
                out=t[64:128, :], in_=self.dram["blob"][g, 64:128, off : off + cb]
            )
        else:
            eng = {"sync": nc.sync, "scalar": nc.scalar, "pool": nc.gpsimd}[
                RING[(g, ch)]
            ]
            eng.dma_start(out=t[:], in_=self.dram["blob"][g, :, off : off + cb])
        self.blob[(g, ch)] = t
        if ch == XCARRIER:
            self.ys[g] = t[:, 2 * w : 2 * w + 1024].bitcast(pools["bf16"])
            self.rtp[g] = t[:, 2 * w + 1024 : 2 * w + 2048].bitcast(pools["f16"])
            self.ident[g] = t[:, 2 * w + 2048 : 2 * w + 2304].bitcast(pools["f16"])

    # ---- compute ---------------------------------------------------------
    def _decode_slice(self, g, ch, adj_a, ea_a, w, name):
        """silu + (s+k)*adj producing a z tile for one decode slice."""
        nc, pools, OP, AF = self.nc, self.pools, self.OP, self.AF
        z = pools["awp"].tile([_P, w], pools["bf16"], name=f"z{name}", tag="z")
        if self.mode in ("silu", "sin"):
            s_t = pools["sp"].tile([_P, w], pools["f16"], name=f"s{name}", tag="s")
            nc.scalar.activation(
                s_t[:], ea_a,
                AF.Silu if self.mode == "silu" else AF.Sin,
                bias=pools["abias_sb"][:, 0:1],
                scale=float(self.params["beta"]),
            )
            if (g, ch) in POOL_MULT:
                nc.vector.tensor_scalar(
                    s_t[:], s_t[:], float(self.params["k"]), None, OP.add
                )
                nc.gpsimd.tensor_tensor(z[:], s_t[:], adj_a, OP.mult)
            else:
                nc.vector.scalar_tensor_tensor(
                    z[:], s_t[:], float(self.params["k"]), adj_a, OP.add, OP.mult
                )
        elif self.mode in ("cubic", "quad"):
            s_t = pools["sp"].tile([_P, w], pools["f16"], name=f"s{name}", tag="s")
            nc.scalar.activation(
                s_t[:], ea_a, AF.Square,
                bias=pools["abias_sb"][:, 0:1], scale=1.0,
            )
            if self.mode == "cubic":
                qt = pools["qtp"].tile([_P, w], pools["f16"], name=f"q{name}", tag="q")
                nc.vector.scalar_tensor_tensor(
                    qt[:], ea_a, float(self.params["r"]), adj_a,
                    OP.subtract, OP.mult,
                )
                nc.vector.scalar_tensor_tensor(
                    z[:], s_t[:], float(self.params["v2"]), qt[:],
                    OP.add, OP.mult,
                )
            else:
                nc.vector.scalar_tensor_tensor(
                    z[:], s_t[:], float(self.params["v2"]), adj_a,
                    OP.add, OP.mult,
                )
        elif self.mode == "linear":
            nc.vector.scalar_tensor_tensor(
                z[:], ea_a, float(self.params["r"]), adj_a,
                OP.subtract, OP.mult,
            )
        else:  # const
            nc.vector.tensor_copy(z[:], adj_a)
        return z

    def seed(self, g):
        """Open the graph's single PSUM group with outT = I.T @ rtp."""
        self.pk[g] = self.pools["ps"].tile(
            [_P, _H], self.pools["f32"], tag=f"pk{g}", name=f"pk{g}"
        )
        self.nc.tensor.matmul(
            self.pk[g][:],
            lhsT=self.ident[g][:, :],
            rhs=self.rtp[g][:, :],
            start=True, stop=False,
            skip_group_check=True,
        )

    def chunk_ops(self, g, ch):
        """Decode slices of DMA chunk ch, each followed by its agg matmuls."""
        nc = self.nc
        wT = CHUNKS[ch][0] * _N
        blob_t = self.blob[(g, ch)]
        if ch == 0 and SPLIT_C0:
            # col-split layout [adjA|eaA|adjB|eaB]: slice h feeds node-half h
            for h in range(2):
                adj_a = blob_t[:, h * _N : h * _N + _H]
                ea_a = blob_t[:, h * _N + _H : (h + 1) * _N]
                z = self._decode_slice(g, ch, adj_a, ea_a, _H, f"{g}_{ch}_{h}")
                nc.tensor.matmul(
                    self.pk[g][h * _C : (h + 1) * _C, :],
                    lhsT=self.ys[g][:, 0:_C],
                    rhs=z[:, :],
                    start=False, stop=False,
                    skip_group_check=True,
                )
            return
        s_off = 0  # tile offset within the DMA chunk
        for si, stiles in enumerate(CHUNKS[ch][1]):
            w = stiles * _N
            adj_a = blob_t[:, s_off * _N : s_off * _N + w]
            ea_a = blob_t[:, wT + s_off * _N : wT + s_off * _N + w]
            z = self._decode_slice(g, ch, adj_a, ea_a, w, f"{g}_{ch}_{si}")
            for sub in range(stiles):
                jt = self.chunk_off[ch] + s_off + sub
                last = jt == self.n_tiles - 1
                for half in range(2):
                    nc.tensor.matmul(
                        self.pk[g][half * _C : (half + 1) * _C, :],
                        lhsT=self.ys[g][:, jt * _C : (jt + 1) * _C],
                        rhs=z[:, sub * _N + half * _H : sub * _N + (half + 1) * _H],
                        start=False, stop=(last and half == 1),
                        skip_group_check=True,
                    )
            s_off += stiles

    def tail_out(self, g):
        """PSUM -> f16 copy split by partition halves (each waits only its
        own half's last matmul; vector+scalar run concurrently), then two
        stores on the sync ring."""
        outb = self.pools["outp"].tile([_P, _H], self.pools["f16"], tag=f"outb{g}")
        self.nc.vector.tensor_copy(outb[0:_C, :], self.pk[g][0:_C, :])
        self.nc.scalar.copy(out=outb[_C:_P, :], in_=self.pk[g][_C:_P, :])
        self.nc.sync.dma_start(out=self.dram["out"][g, 0:_C, :], in_=outb[0:_C, :])
        self.nc.sync.dma_start(out=self.dram["out"][g, _C:_P, :], in_=outb[_C:_P, :])


def _build_module(mode, params):
    import concourse.bass as bass  # noqa: F401
    from concourse import bacc, mybir
    from concourse.tile import TileContext

    f32 = mybir.dt.float32
    f16 = mybir.dt.float16
    bf16 = mybir.dt.bfloat16
    u8 = mybir.dt.uint8

    nc = bacc.Bacc(
        "TRN2", target_bir_lowering=False, debug=False, num_devices=_NCORES
    )

    blob_w = _NT * _N * 2 + 2304
    dram = {
        "blob": nc.dram_tensor("blob", [_G, _P, blob_w], u8, kind="ExternalInput"),
        "out": nc.dram_tensor("out", [_G, _P, _H], f16, kind="ExternalOutput"),
    }

    pool_specs = [
        ("consts", 1, None),
        ("blobp", 1, None),
        ("sp", SP_BUFS, None),
        ("awp", AW_BUFS, None),
        ("outp", 1, None),
        ("ps", 1, "PSUM"),
    ]
    if mode == "cubic":
        pool_specs.insert(3, ("qtp", 2, None))

    with TileContext(nc) as tc, ExitStack() as ctx:
        pools = {"f32": f32, "f16": f16, "bf16": bf16, "u8": u8, "tc": tc}
        for name, bufs, space in pool_specs:
            kw = {"space": space} if space else {}
            pools[name] = ctx.enter_context(tc.tile_pool(name=name, bufs=bufs, **kw))

        prio = tc.high_priority() if PRIO_CONSTS else None
        if prio:
            prio.__enter__()
        if mode in ("silu", "sin", "cubic", "quad"):
            ab = pools["consts"].tile([_P, 1], f32, tag="abias")
            bias_val = params["gamma"] if mode in ("silu", "sin") else params["h"]
            nc.vector.memset(ab[:], float(bias_val))
            pools["abias_sb"] = ab

        if mode in ("silu", "sin"):
            # dummy activation exactly mirroring the real one (u8 input,
            # f16 out, AP bias + float scale) as the FIRST Scalar
            # instruction: pulls the ACT_TABLE_LOAD into startup dead-time.
            dmu = pools["consts"].tile([_P, 1], u8, tag="dummy_in")
            nc.gpsimd.memset(dmu[:], 0)
            dout = pools["consts"].tile([_P, 1], f16, tag="dummy_out")
            af = (
                mybir.ActivationFunctionType.Silu
                if mode == "silu"
                else mybir.ActivationFunctionType.Sin
            )
            nc.scalar.activation(
                dout[:], dmu[:], af, bias=ab[:, 0:1],
                scale=float(params["beta"]),
            )
        if prio:
            prio.__exit__(None, None, None)

        em = _Emitter(nc, pools, dram, mode, params)

        # ---- all input DMA issues in execution order at priority 0 so the
        # 8 HWDGE semaphore lanes recycle without false serialization.
        with tc.high_priority():
            for item in ISSUE_ORDER:
                em.issue_blob(*item)

        for g in range(_G):
            em.seed(g)
            for ch in range(len(CHUNKS)):
                em.chunk_ops(g, ch)
            em.tail_out(g)

    nc.finalize()
    return nc


def _get_module(w_edge):
    mode, params, lead = _fit_chain(w_edge)
    key = (
        mode,
        tuple(sorted((k, round(v, 15)) for k, v in params.items())),
        tuple((t, tuple(s)) for t, s in CHUNKS),
        tuple(sorted(RING.items())),
        tuple(POOL_MULT),
        tuple(map(str, ISSUE_ORDER)),
        STRIPE,
        PRIO_CONSTS,
        SP_BUFS,
        AW_BUFS,
        SPLIT_C0,
        XCARRIER,
    )
    if key not in _BUILD_CACHE:
        _BUILD_CACHE[key] = _build_module(mode, params)
    return _BUILD_CACHE[key], lead


def kernel(x, adj, edge_attr, W_rel, b_rel, W_root, w_edge):
    global LAST_RESULTS
    from concourse import mybir
    from concourse.bass_utils import run_bass_kernel_spmd

    bf16np = mybir.dt.np(mybir.dt.bfloat16)
    f16np = mybir.dt.np(mybir.dt.float16)

    x = np.asarray(x, dtype=np.float32)
    adj = np.asarray(adj, dtype=np.float32)
    ea = np.asarray(edge_attr, dtype=np.int32).reshape(_B, _N, _N)
    W_rel = np.asarray(W_rel, dtype=np.float64)
    W_root = np.asarray(W_root, dtype=np.float64)
    b_rel = np.asarray(b_rel, dtype=np.float64).reshape(1, _C)
    w_edge = np.asarray(w_edge)

    nc, lead = _get_module(w_edge)

    def tile_rows(a):
        """[B, N, F] -> [B, 128, NT*F]: row j*128+p of graph b lands at
        [b, p, j*F:(j+1)*F] -- one contiguous free-dim line per partition."""
        B, N, F = a.shape
        return np.ascontiguousarray(
            a.reshape(B, _NT, _P, F).transpose(0, 2, 1, 3).reshape(B, _P, _NT * F)
        )

    # transposed + row-tiled layouts; adj quantized to u8 (x255)
    adjq = np.rint(
        tile_rows(np.ascontiguousarray(adj.transpose(0, 2, 1))) * 255.0
    ).astype(np.uint8)
    eaT = tile_rows(np.ascontiguousarray(ea.transpose(0, 2, 1))).astype(np.uint8)

    # associativity folds: ys = x @ (lead/255 * W_rel); rt = x @ W_root + b_rel
    x64 = x.astype(np.float64)
    ys = tile_rows((x64 @ (lead / 255.0 * W_rel)).astype(np.float32)).astype(bf16np)
    rt = (x64 @ W_root + b_rel).astype(np.float32)  # [B, N, C]
    # rtp[g, h*64+c, i] = rt[g, h*512+i, c]
    rtp = np.ascontiguousarray(
        rt.reshape(_B, 2, _H, _C).transpose(0, 1, 3, 2).reshape(_B, _P, _H)
    ).astype(f16np)
    xpack = np.ascontiguousarray(
        np.concatenate([ys.view(np.uint8), rtp.view(np.uint8)], axis=2)
    )
    ident = np.eye(_P, dtype=f16np)
    blob = _pack_blob(adjq, eaT, xpack, ident)

    in_maps = []
    for c in range(_NCORES):
        sl = slice(c * _G, (c + 1) * _G)
        in_maps.append({"blob": blob[sl]})

    res = run_bass_kernel_spmd(nc, in_maps, list(range(_NCORES)), trace=TRACE)
    LAST_RESULTS = res
    # out staged [G, 128, 512] f16: out[g, h*512+i, c] = outs[g, h*64+c, i]
    outs = np.concatenate(
        [np.asarray(res.results[c]["out"]) for c in range(_NCORES)], axis=0
    ).astype(np.float32)
    out = (
        outs.reshape(_B, 2, _C, _H)
        .transpose(0, 1, 3, 2)
        .reshape(_B, _N, _C)
    )
    return np.ascontiguousarray(out)


# revision 79
# speedup vs baseline: 1.0411x; 1.0404x over previous
"""DenseGATConv-style GNN message passing kernel for Trainium2 (Bass/Tile).

Math (per graph b):
    e      = w_edge[edge_attr[b]]            # [N, N] gather from 4-entry table
    adj_w  = adj[b] * e                      # weighted adjacency
    out[b] = adj_w @ x[b] @ W_rel + b_rel + x[b] @ W_root

v4 design (v2 was 45us; v3 traced the walls: ScalarE silu spine ~14.5us,
DVE multiply ~18us, 8 HWDGE semaphore lanes serializing DMA issues):
  * ASSOCIATIVITY: (adj_w @ x) @ W_rel == adj_w @ (x @ W_rel).  The host
    pre-multiplies ys = x @ (lead*W_rel) and rt = x @ W_root + b_rel
    (0.3% of FLOPs); the device aggregation THEN IS the output:
        outT = ysT-contraction over the weighted adjacency, seeded with
        rtT via an identity-lhsT matmul into the same PSUM group.
    No stacked tile, no tail transform, no second x copy.
  * adj ships uint8 (round(adj*255); 1/255 folds into ys), edge_attr
    uint8; decode chain per chunk:
        s = Silu(beta*ea+gamma) [ACT, u8-in]; z = (s+k)*adj_u8 [DVE stt]
    For two middle chunks the multiply runs on the Pool engine instead
    (DVE computes s+k at 4x, Pool does the TT multiply) to balance DVE.
  * DMA: chunks [1,2,2,2,1] per graph alternate the two HWDGE rings;
    issues are emitted in execution order at high priority so the 8
    HWDGE semaphore lanes recycle without false serialization.
  * Output: PSUM [128,512] f32 per graph (both node-halves packed on the
    partition axis via tile_position), one start=True seed, copied to
    f16 and stored transposed; host untransposes.

Sharding: data-parallel over batch B=16 across 8 cores (2 graphs/core);
weights replicated.
"""

import sys
from contextlib import ExitStack

sys.path.insert(0, "/opt/trn_rl_repo")

import numpy as np

_B, _N, _C = 16, 1024, 64
_NCORES = 8
_G = _B // _NCORES  # graphs per core
_P = 128
_NT = _N // _P  # 128-row tiles per graph
_H = 512  # half-graph columns (one PSUM bank of fp32)

# ---- schedule knobs (test.py may override before calling kernel()) ----
TRACE = False
# per-graph DMA chunks: (dma_tiles, [decode slice tile counts]) -- DMA
# granularity (descriptor/line size) decoupled from compute granularity.
# Small first chunk starts the silu spine early; fat rest for queue BW.
CHUNKS = [(1, [1]), (1, [1]), (2, [2]), (2, [2]), (2, [1, 1])]
# ring per (g, ch): alternate, g1 phase-shifted; the two lane-recycled
# issues (emitted last) go to sync so they never block Scalar's silus.
RING = {(0, 0): "sync", (0, 1): "scalar", (0, 2): "sync", (0, 3): "scalar",
        (0, 4): "sync", (1, 0): "scalar", (1, 1): "sync", (1, 2): "scalar",
        (1, 3): "sync", (1, 4): "sync"}
# chunks whose (s+k)*adj multiply runs on Pool (DVE does s+=k at 4x first).
# NOTE: measured HARMFUL -- concurrent Pool TT slows DVE stt 2.3x (SBUF
# contention); keep empty.
POOL_MULT = []
# DMA emission order = HWDGE semaphore lane assignment = execution order.
ISSUE_ORDER = [(0, 0), (0, 1), (0, 2), (0, 3), (0, 4), (1, 0), (1, 1), (1, 2),
               (1, 3), (1, 4)]
# stripe each chunk across both HWDGE rings by partition halves (requires
# <= 4 chunks total so the 8 semaphore lanes aren't recycled by loads)
STRIPE = False
# emit consts/dummy at priority 0 too: Scalar runs table-load+dummy BEFORE
# its DMA issues, so the first silu isn't stuck behind the issue stream
PRIO_CONSTS = True
# tile-pool ring depths: deeper s/z rings let ACT run ahead of DVE and
# absorb DMA arrival jitter
SP_BUFS = 6
AW_BUFS = 8
# chunk 0 packs column-split halves; graph 0 loads them as TWO descriptors
# on BOTH rings in parallel (first data ~1us earlier -> earlier spine start)
SPLIT_C0 = False
# which DMA chunk carries the xpack/ident payload (PE-only consumers can
# tolerate late arrival; the decode spine cannot)
XCARRIER = 1
LAST_RESULTS = None

_BUILD_CACHE = {}


def _pack_blob(adjq, eaT, xpack, ident):
    """Chunk-major byte blob: per DMA chunk [adj_u8 w | ea_u8 w]; chunk 1
    additionally carries [xpack 2048B] (ys+rtp) and [ident 256B] -- keeps
    total load DMAs at 8 (= HWDGE semaphore lanes, so no recycling)."""
    B = adjq.shape[0]
    id_rows = np.broadcast_to(
        ident.view(np.uint8).reshape(1, _P, 2 * _P), (B, _P, 2 * _P)
    )
    segs = []
    off = 0
    for ch, (ctiles, _slices) in enumerate(CHUNKS):
        w = ctiles * _N
        if ch == 0 and SPLIT_C0:
            # column-split halves: [adjA|eaA|adjB|eaB], 512 cols each
            hw = w // 2
            segs.append(adjq[:, :, off : off + hw])
            segs.append(eaT[:, :, off : off + hw])
            segs.append(adjq[:, :, off + hw : off + w])
            segs.append(eaT[:, :, off + hw : off + w])
        else:
            segs.append(adjq[:, :, off : off + w])
            segs.append(eaT[:, :, off : off + w])
        if ch == XCARRIER:
            segs.append(xpack)
            segs.append(id_rows)
        off += w
    return np.ascontiguousarray(np.concatenate(segs, axis=2))


def _poly_coeffs(w_edge):
    w = np.asarray(w_edge, dtype=np.float64).reshape(4)
    V = np.vander(np.arange(4.0), 4, increasing=True)
    return np.linalg.solve(V, w)


def _act_fit(f, w, n_starts=6000, seed=0):
    """Exact 4-point fit w[a] = alpha*f(beta*a+gamma)+delta via random-start
    Gauss-Newton (numpy only).  Returns (beta, gamma, alpha, delta) or None."""
    w = np.asarray(w, dtype=np.float64).reshape(4)
    a4 = np.arange(4.0)
    scale = max(np.max(np.abs(w)), 1e-30)
    rng = np.random.default_rng(seed)
    best = None
    for _ in range(n_starts):
        b = rng.uniform(-4.0, 4.0)
        g = rng.uniform(-8.0, 8.0)
        M = np.stack([f(b * a4 + g), np.ones(4)], axis=1)
        sol, *_ = np.linalg.lstsq(M, w, rcond=None)
        r = M @ sol - w
        v = float(r @ r)
        if best is None or v < best[0]:
            best = (v, b, g, float(sol[0]), float(sol[1]))
    p = np.array(best[1:], dtype=np.float64)
    eps = 1e-6
    for _ in range(200):
        b, g, al, de = p
        r = al * f(b * a4 + g) + de - w
        if np.abs(r).max() < 1e-12 * scale:
            break
        J = np.empty((4, 4))
        for j in range(4):
            q = p.copy()
            q[j] += eps
            J[:, j] = (q[2] * f(q[0] * a4 + q[1]) + q[3] - w - r) / eps
        try:
            step, *_ = np.linalg.lstsq(J, r, rcond=None)
        except np.linalg.LinAlgError:
            return None
        p = p - step
    b, g, al, de = p
    r = al * f(b * a4 + g) + de - w
    if np.abs(r).max() < 1e-9 * scale and abs(al) > 1e-9 * scale:
        return float(b), float(g), float(al), float(de)
    return None


def _fit_chain(w_edge):
    """Pick the device chain for e = w_edge[a], a in {0..3}.

    Preferred: exact silu fit  e = alpha*silu(beta*a+gamma) + delta
    (one ScalarE activation + one DVE stt), then sin (domain checked).
    Falls back to the factored cubic (one ScalarE Square + 2 DVE stt).

    Returns (mode, params, lead): device computes z = chain(a) * adj_u8
    such that true adj_w = (lead/255) * z; lead/255 folds into ys.
    """
    w = np.asarray(w_edge, dtype=np.float64).reshape(4)
    v0, v1, v2, v3 = w
    scale = max(np.max(np.abs(w)), 1e-30)

    def silu(x):
        return x / (1.0 + np.exp(-np.clip(x, -60, 60)))

    fit = _act_fit(silu, w)
    if fit is not None and abs(fit[0]) * 3 + abs(fit[1]) < 30.0:
        b, g, al, de = fit
        return "silu", dict(beta=b, gamma=g, k=float(de / al)), al

    den = (v0 + v2) + 2.0 * v2 - (v1 + v3) - 2.0 * v1
    if abs(den) > 1e-9 * scale:
        d = ((v0 + v2) * v2 - (v1 + v3) * v1) / den
        if abs(v1 - d) > 1e-9 * scale:
            c = (v0 + v2 - 2.0 * d) / (2.0 * (v1 - d))
            if abs(c) < 1.0 - 1e-7:
                b = float(np.arccos(c))
                sb = np.sin(b)
                Pv = v0 - d
                Qv = ((v1 - d) - Pv * c) / sb
                alpha = float(np.hypot(Pv, Qv))
                g = float(np.arctan2(Pv, Qv))
                args = b * np.arange(4.0) + g
                fitv = alpha * np.sin(args) + d
                if (
                    np.abs(fitv - w).max() < 1e-9 * scale
                    and alpha > 1e-9 * scale
                    and np.abs(args).max() <= np.pi
                ):
                    return ("sin", dict(beta=b, gamma=g, k=float(d / alpha)), alpha)

    c0, c1, c2, c3 = _poly_coeffs(w)
    tol = 1e-7 * scale
    if abs(c3) > tol:
        A, Bc, Cc = c2 / c3, c1 / c3, c0 / c3
        roots = np.roots([1.0, A, Bc, Cc])
        r = float(np.real(roots[np.argmin(np.abs(np.imag(roots)))]))
        p = A + r
        q = Bc + p * r
        return "cubic", dict(r=r, h=p / 2.0, v2=q - p * p / 4.0), c3
    if abs(c2) > tol:
        p2, q2 = c1 / c2, c0 / c2
        return "quad", dict(h=p2 / 2.0, v2=q2 - p2 * p2 / 4.0), c2
    if abs(c1) > tol:
        return "linear", dict(r=-c0 / c1), c1
    return "const", dict(), c0


class _Emitter:
    """Holds build state; emits device ops in explicit global order."""

    def __init__(self, nc, pools, dram, mode, params):
        from concourse import mybir

        self.nc = nc
        self.pools = pools
        self.dram = dram
        self.mode = mode
        self.params = params
        self.OP = mybir.AluOpType
        self.AF = mybir.ActivationFunctionType
        self.blob = {}  # (g, ch) -> blob tile
        self.ys = {}
        self.rtp = {}
        self.ident = {}
        self.pk = {}  # g -> packed psum accumulator [128, 512]
        self.chunk_off = np.cumsum([0] + [c[0] for c in CHUNKS[:-1]])
        self.n_tiles = sum(c[0] for c in CHUNKS)

    # ---- DMA issues ------------------------------------------------------
    def issue_blob(self, g, ch):
        nc, pools = self.nc, self.pools
        w = CHUNKS[ch][0] * _N
        cb = 2 * w + (2304 if ch == XCARRIER else 0)
        off = 2 * self.chunk_off[ch] * _N + (2304 if ch > XCARRIER else 0)
        t = pools["blobp"].tile(
            [_P, cb], pools["u8"], name=f"blob{g}_{ch}", tag=f"blob{ch}", bufs=2
        )
        if ch == 0 and SPLIT_C0 and g == 0:
            # column-split halves land via BOTH rings in parallel
            hb = cb // 2
            nc.sync.dma_start(
                out=t[:, 0:hb], in_=self.dram["blob"][g, :, off : off + hb]
            )
            nc.scalar.dma_start(
                out=t[:, hb:cb], in_=self.dram["blob"][g, :, off + hb : off + cb]
            )
        elif STRIPE:
            # partition-halved across BOTH HWDGE rings: 2x arrival speed,
            # queues stay byte-balanced with no ring assignment tuning.
            nc.sync.dma_start(
                out=t[0:64, :], in_=self.dram["blob"][g, 0:64, off : off + cb]
            )
            nc.scalar.dma_start(
                out=t[64:128, :], in_=self.dram["blob"][g, 64:128, off : off + cb]
            )
        else:
            eng = {"sync": nc.sync, "scalar": nc.scalar, "pool": nc.gpsimd}[
                RING[(g, ch)]
            ]
            eng.dma_start(out=t[:], in_=self.dram["blob"][g, :, off : off + cb])
        self.blob[(g, ch)] = t
        if ch == XCARRIER:
            self.ys[g] = t[:, 2 * w : 2 * w + 1024].bitcast(pools["bf16"])
            self.rtp[g] = t[:, 2 * w + 1024 : 2 * w + 2048].bitcast(pools["f16"])
            self.ident[g] = t[:, 2 * w + 2048 : 2 * w + 2304].bitcast(pools["f16"])

    # ---- compute ---------------------------------------------------------
    def _decode_slice(self, g, ch, adj_a, ea_a, w, name):
        """silu + (s+k)*adj producing a z tile for one decode slice."""
        nc, pools, OP, AF = self.nc, self.pools, self.OP, self.AF
        z = pools["awp"].tile([_P, w], pools["bf16"], name=f"z{name}", tag="z")
        if self.mode in ("silu", "sin"):
            s_t = pools["sp"].tile([_P, w], pools["f16"], name=f"s{name}", tag="s")
            nc.scalar.activation(
                s_t[:], ea_a,
                AF.Silu if self.mode == "silu" else AF.Sin,
                bias=pools["abias_sb"][:, 0:1],
                scale=float(self.params["beta"]),
            )
            if (g, ch) in POOL_MULT:
                nc.vector.tensor_scalar(
                    s_t[:], s_t[:], float(self.params["k"]), None, OP.add
                )
                nc.gpsimd.tensor_tensor(z[:], s_t[:], adj_a, OP.mult)
            else:
                nc.vector.scalar_tensor_tensor(
                    z[:], s_t[:], float(self.params["k"]), adj_a, OP.add, OP.mult
                )
        elif self.mode in ("cubic", "quad"):
            s_t = pools["sp"].tile([_P, w], pools["f16"], name=f"s{name}", tag="s")
            nc.scalar.activation(
                s_t[:], ea_a, AF.Square,
                bias=pools["abias_sb"][:, 0:1], scale=1.0,
            )
            if self.mode == "cubic":
                qt = pools["qtp"].tile([_P, w], pools["f16"], name=f"q{name}", tag="q")
                nc.vector.scalar_tensor_tensor(
                    qt[:], ea_a, float(self.params["r"]), adj_a,
                    OP.subtract, OP.mult,
                )
                nc.vector.scalar_tensor_tensor(
                    z[:], s_t[:], float(self.params["v2"]), qt[:],
                    OP.add, OP.mult,
                )
            else:
                nc.vector.scalar_tensor_tensor(
                    z[:], s_t[:], float(self.params["v2"]), adj_a,
                    OP.add, OP.mult,
                )
        elif self.mode == "linear":
            nc.vector.scalar_tensor_tensor(
                z[:], ea_a, float(self.params["r"]), adj_a,
                OP.subtract, OP.mult,
            )
        else:  # const
            nc.vector.tensor_copy(z[:], adj_a)
        return z

    def seed(self, g):
        """Open the graph's single PSUM group with outT = I.T @ rtp."""
        self.pk[g] = self.pools["ps"].tile(
            [_P, _H], self.pools["f32"], tag=f"pk{g}", name=f"pk{g}"
        )
        self.nc.tensor.matmul(
            self.pk[g][:],
            lhsT=self.ident[g][:, :],
            rhs=self.rtp[g][:, :],
            start=True, stop=False,
            skip_group_check=True,
        )

    def chunk_ops(self, g, ch):
        """Decode slices of DMA chunk ch, each followed by its agg matmuls."""
        nc = self.nc
        wT = CHUNKS[ch][0] * _N
        blob_t = self.blob[(g, ch)]
        if ch == 0 and SPLIT_C0:
            # col-split layout [adjA|eaA|adjB|eaB]: slice h feeds node-half h
            for h in range(2):
                adj_a = blob_t[:, h * _N : h * _N + _H]
                ea_a = blob_t[:, h * _N + _H : (h + 1) * _N]
                z = self._decode_slice(g, ch, adj_a, ea_a, _H, f"{g}_{ch}_{h}")
                nc.tensor.matmul(
                    self.pk[g][h * _C : (h + 1) * _C, :],
                    lhsT=self.ys[g][:, 0:_C],
                    rhs=z[:, :],
                    start=False, stop=False,
                    skip_group_check=True,
                )
            return
        s_off = 0  # tile offset within the DMA chunk
        for si, stiles in enumerate(CHUNKS[ch][1]):
            w = stiles * _N
            adj_a = blob_t[:, s_off * _N : s_off * _N + w]
            ea_a = blob_t[:, wT + s_off * _N : wT + s_off * _N + w]
            z = self._decode_slice(g, ch, adj_a, ea_a, w, f"{g}_{ch}_{si}")
            for sub in range(stiles):
                jt = self.chunk_off[ch] + s_off + sub
                last = jt == self.n_tiles - 1
                for half in range(2):
                    nc.tensor.matmul(
                        self.pk[g][half * _C : (half + 1) * _C, :],
                        lhsT=self.ys[g][:, jt * _C : (jt + 1) * _C],
                        rhs=z[:, sub * _N + half * _H : sub * _N + (half + 1) * _H],
                        start=False, stop=(last and half == 1),
                        skip_group_check=True,
                    )
            s_off += stiles

    def tail_out(self, g):
        """PSUM -> f16 copy split by partition halves (each waits only its
        own half's last matmul; vector+scalar run concurrently), then two
        stores on the sync ring."""
        outb = self.pools["outp"].tile([_P, _H], self.pools["f16"], tag=f"outb{g}")
        self.nc.vector.tensor_copy(outb[0:_C, :], self.pk[g][0:_C, :])
        self.nc.scalar.copy(out=outb[_C:_P, :], in_=self.pk[g][_C:_P, :])
        self.nc.sync.dma_start(out=self.dram["out"][g, 0:_C, :], in_=outb[0:_C, :])
        self.nc.sync.dma_start(out=self.dram["out"][g, _C:_P, :], in_=outb[_C:_P, :])


def _build_module(mode, params):
    import concourse.bass as bass  # noqa: F401
    from concourse import bacc, mybir
    from concourse.tile import TileContext

    f32 = mybir.dt.float32
    f16 = mybir.dt.float16
    bf16 = mybir.dt.bfloat16
    u8 = mybir.dt.uint8

    nc = bacc.Bacc(
        "TRN2", target_bir_lowering=False, debug=False, num_devices=_NCORES
    )

    blob_w = _NT * _N * 2 + 2304
    dram = {
        "blob": nc.dram_tensor("blob", [_G, _P, blob_w], u8, kind="ExternalInput"),
        "out": nc.dram_tensor("out", [_G, _P, _H], f16, kind="ExternalOutput"),
    }

    pool_specs = [
        ("consts", 1, None),
        ("blobp", 1, None),
        ("sp", SP_BUFS, None),
        ("awp", AW_BUFS, None),
        ("outp", 1, None),
        ("ps", 1, "PSUM"),
    ]
    if mode == "cubic":
        pool_specs.insert(3, ("qtp", 2, None))

    with TileContext(nc) as tc, ExitStack() as ctx:
        pools = {"f32": f32, "f16": f16, "bf16": bf16, "u8": u8, "tc": tc}
        for name, bufs, space in pool_specs:
            kw = {"space": space} if space else {}
            pools[name] = ctx.enter_context(tc.tile_pool(name=name, bufs=bufs, **kw))

        prio = tc.high_priority() if PRIO_CONSTS else None
        if prio:
            prio.__enter__()
        if mode in ("silu", "sin", "cubic", "quad"):
            ab = pools["consts"].tile([_P, 1], f32, tag="abias")
            bias_val = params["gamma"] if mode in ("silu", "sin") else params["h"]
            nc.vector.memset(ab[:], float(bias_val))
            pools["abias_sb"] = ab

        if mode in ("silu", "sin"):
            # dummy activation exactly mirroring the real one (u8 input,
            # f16 out, AP bias + float scale) as the FIRST Scalar
            # instruction: pulls the ACT_TABLE_LOAD into startup dead-time.
            dmu = pools["consts"].tile([_P, 1], u8, tag="dummy_in")
            nc.gpsimd.memset(dmu[:], 0)
            dout = pools["consts"].tile([_P, 1], f16, tag="dummy_out")
            af = (
                mybir.ActivationFunctionType.Silu
                if mode == "silu"
                else mybir.ActivationFunctionType.Sin
            )
            nc.scalar.activation(
                dout[:], dmu[:], af, bias=ab[:, 0:1],
                scale=float(params["beta"]),
            )
        if prio:
            prio.__exit__(None, None, None)

        em = _Emitter(nc, pools, dram, mode, params)

        # ---- all input DMA issues in execution order at priority 0 so the
        # 8 HWDGE semaphore lanes recycle without false serialization.
        with tc.high_priority():
            for item in ISSUE_ORDER:
                em.issue_blob(*item)

        for g in range(_G):
            em.seed(g)
            for ch in range(len(CHUNKS)):
                em.chunk_ops(g, ch)
            em.tail_out(g)

    nc.finalize()
    return nc


def _get_module(w_edge):
    mode, params, lead = _fit_chain(w_edge)
    key = (
        mode,
        tuple(sorted((k, round(v, 15)) for k, v in params.items())),
        tuple((t, tuple(s)) for t, s in CHUNKS),
        tuple(sorted(RING.items())),
        tuple(POOL_MULT),
        tuple(map(str, ISSUE_ORDER)),
        STRIPE,
        PRIO_CONSTS,
        SP_BUFS,
        AW_BUFS,
        SPLIT_C0,
        XCARRIER,
    )
    if key not in _BUILD_CACHE:
        _BUILD_CACHE[key] = _build_module(mode, params)
    return _BUILD_CACHE[key], lead


def kernel(x, adj, edge_attr, W_rel, b_rel, W_root, w_edge):
    global LAST_RESULTS
    from concourse import mybir
    from concourse.bass_utils import run_bass_kernel_spmd

    bf16np = mybir.dt.np(mybir.dt.bfloat16)
    f16np = mybir.dt.np(mybir.dt.float16)

    x = np.asarray(x, dtype=np.float32)
    adj = np.asarray(adj, dtype=np.float32)
    ea = np.asarray(edge_attr, dtype=np.int32).reshape(_B, _N, _N)
    W_rel = np.asarray(W_rel, dtype=np.float64)
    W_root = np.asarray(W_root, dtype=np.float64)
    b_rel = np.asarray(b_rel, dtype=np.float64).reshape(1, _C)
    w_edge = np.asarray(w_edge)

    nc, lead = _get_module(w_edge)

    def tile_rows(a):
        """[B, N, F] -> [B, 128, NT*F]: row j*128+p of graph b lands at
        [b, p, j*F:(j+1)*F] -- one contiguous free-dim line per partition."""
        B, N, F = a.shape
        return np.ascontiguousarray(
            a.reshape(B, _NT, _P, F).transpose(0, 2, 1, 3).reshape(B, _P, _NT * F)
        )

    # transposed + row-tiled layouts; adj quantized to u8 (x255)
    adjq = np.rint(
        tile_rows(np.ascontiguousarray(adj.transpose(0, 2, 1))) * 255.0
    ).astype(np.uint8)
    eaT = tile_rows(np.ascontiguousarray(ea.transpose(0, 2, 1))).astype(np.uint8)

    # associativity folds: ys = x @ (lead/255 * W_rel); rt = x @ W_root + b_rel
    x64 = x.astype(np.float64)
    ys = tile_rows((x64 @ (lead / 255.0 * W_rel)).astype(np.float32)).astype(bf16np)
    rt = (x64 @ W_root + b_rel).astype(np.float32)  # [B, N, C]
    # rtp[g, h*64+c, i] = rt[g, h*512+i, c]
    rtp = np.ascontiguousarray(
        rt.reshape(_B, 2, _H, _C).transpose(0, 1, 3, 2).reshape(_B, _P, _H)
    ).astype(f16np)
    xpack = np.ascontiguousarray(
        np.concatenate([ys.view(np.uint8), rtp.view(np.uint8)], axis=2)
    )
    ident = np.eye(_P, dtype=f16np)
    blob = _pack_blob(adjq, eaT, xpack, ident)

    in_maps = []
    for c in range(_NCORES):
        sl = slice(c * _G, (c + 1) * _G)
        in_maps.append({"blob": blob[sl]})

    res = run_bass_kernel_spmd(nc, in_maps, list(range(_NCORES)), trace=TRACE)
    LAST_RESULTS = res
    # out staged [G, 128, 512] f16: out[g, h*512+i, c] = outs[g, h*64+c, i]
    outs = np.concatenate(
        [np.asarray(res.results[c]["out"]) for c in range(_NCORES)], axis=0
    ).astype(np.float32)
    out = (
        outs.reshape(_B, 2, _C, _H)
        .transpose(0, 1, 3, 2)
        .reshape(_B, _N, _C)
    )
    return np.ascontiguousarray(out)


# revision 82
# speedup vs baseline: 1.0623x; 1.0204x over previous
"""DenseGATConv-style GNN message passing kernel for Trainium2 (Bass/Tile).

Math (per graph b):
    e      = w_edge[edge_attr[b]]            # [N, N] gather from 4-entry table
    adj_w  = adj[b] * e                      # weighted adjacency
    out[b] = adj_w @ x[b] @ W_rel + b_rel + x[b] @ W_root

v4 design (v2 was 45us; v3 traced the walls: ScalarE silu spine ~14.5us,
DVE multiply ~18us, 8 HWDGE semaphore lanes serializing DMA issues):
  * ASSOCIATIVITY: (adj_w @ x) @ W_rel == adj_w @ (x @ W_rel).  The host
    pre-multiplies ys = x @ (lead*W_rel) and rt = x @ W_root + b_rel
    (0.3% of FLOPs); the device aggregation THEN IS the output:
        outT = ysT-contraction over the weighted adjacency, seeded with
        rtT via an identity-lhsT matmul into the same PSUM group.
    No stacked tile, no tail transform, no second x copy.
  * adj ships uint8 (round(adj*255); 1/255 folds into ys), edge_attr
    uint8; decode chain per chunk:
        s = Silu(beta*ea+gamma) [ACT, u8-in]; z = (s+k)*adj_u8 [DVE stt]
    For two middle chunks the multiply runs on the Pool engine instead
    (DVE computes s+k at 4x, Pool does the TT multiply) to balance DVE.
  * DMA: chunks [1,2,2,2,1] per graph alternate the two HWDGE rings;
    issues are emitted in execution order at high priority so the 8
    HWDGE semaphore lanes recycle without false serialization.
  * Output: PSUM [128,512] f32 per graph (both node-halves packed on the
    partition axis via tile_position), one start=True seed, copied to
    f16 and stored transposed; host untransposes.

Sharding: data-parallel over batch B=16 across 8 cores (2 graphs/core);
weights replicated.
"""

import sys
from contextlib import ExitStack

sys.path.insert(0, "/opt/trn_rl_repo")

import numpy as np

_B, _N, _C = 16, 1024, 64
_NCORES = 8
_G = _B // _NCORES  # graphs per core
_P = 128
_NT = _N // _P  # 128-row tiles per graph
_H = 512  # half-graph columns (one PSUM bank of fp32)

# ---- schedule knobs (test.py may override before calling kernel()) ----
TRACE = False
# per-graph DMA chunks: (dma_tiles, [decode slice tile counts]) -- DMA
# granularity (descriptor/line size) decoupled from compute granularity.
# Small first chunk starts the silu spine early; fat rest for queue BW.
CHUNKS = [(1, [1]), (1, [1]), (2, [2]), (2, [2]), (2, [1, 1])]
# ring per (g, ch): alternate, g1 phase-shifted; the two lane-recycled
# issues (emitted last) go to sync so they never block Scalar's silus.
RING = {(0, 0): "sync", (0, 1): "scalar", (0, 2): "sync", (0, 3): "scalar",
        (0, 4): "sync", (1, 0): "scalar", (1, 1): "sync", (1, 2): "scalar",
        (1, 3): "sync", (1, 4): "sync"}
# chunks whose (s+k)*adj multiply runs on Pool (DVE does s+=k at 4x first).
# NOTE: measured HARMFUL -- concurrent Pool TT slows DVE stt 2.3x (SBUF
# contention); keep empty.
POOL_MULT = []
# DMA emission order = HWDGE semaphore lane assignment = execution order.
ISSUE_ORDER = [(0, 0), (0, 1), (0, 2), (0, 3), (0, 4), (1, 0), (1, 1), (1, 2),
               (1, 3), (1, 4)]
# stripe each chunk across both HWDGE rings by partition halves (requires
# <= 4 chunks total so the 8 semaphore lanes aren't recycled by loads)
STRIPE = False
# emit consts/dummy at priority 0 too: Scalar runs table-load+dummy BEFORE
# its DMA issues, so the first silu isn't stuck behind the issue stream
PRIO_CONSTS = True
# tile-pool ring depths: deeper s/z rings let ACT run ahead of DVE and
# absorb DMA arrival jitter
SP_BUFS = 6
AW_BUFS = 8
# chunk 0 packs column-split halves; graph 0 loads them as TWO descriptors
# on BOTH rings in parallel (first data ~1us earlier -> earlier spine start)
SPLIT_C0 = False
# which DMA chunk carries the xpack/ident payload (PE-only consumers can
# tolerate late arrival; the decode spine cannot)
XCARRIER = 1
# split the FINAL decode slice's stt into column halves so the h0 matmul
# overlaps the h1 stt (hides ~0.6us of tail)
SPLIT_TAIL = True
LAST_RESULTS = None

_BUILD_CACHE = {}


def _pack_blob(adjq, eaT, xpack, ident):
    """Chunk-major byte blob: per DMA chunk [adj_u8 w | ea_u8 w]; chunk 1
    additionally carries [xpack 2048B] (ys+rtp) and [ident 256B] -- keeps
    total load DMAs at 8 (= HWDGE semaphore lanes, so no recycling)."""
    B = adjq.shape[0]
    id_rows = np.broadcast_to(
        ident.view(np.uint8).reshape(1, _P, 2 * _P), (B, _P, 2 * _P)
    )
    segs = []
    off = 0
    for ch, (ctiles, _slices) in enumerate(CHUNKS):
        w = ctiles * _N
        if ch == 0 and SPLIT_C0:
            # column-split halves: [adjA|eaA|adjB|eaB], 512 cols each
            hw = w // 2
            segs.append(adjq[:, :, off : off + hw])
            segs.append(eaT[:, :, off : off + hw])
            segs.append(adjq[:, :, off + hw : off + w])
            segs.append(eaT[:, :, off + hw : off + w])
        else:
            segs.append(adjq[:, :, off : off + w])
            segs.append(eaT[:, :, off : off + w])
        if ch == XCARRIER:
            segs.append(xpack)
            segs.append(id_rows)
        off += w
    return np.ascontiguousarray(np.concatenate(segs, axis=2))


def _poly_coeffs(w_edge):
    w = np.asarray(w_edge, dtype=np.float64).reshape(4)
    V = np.vander(np.arange(4.0), 4, increasing=True)
    return np.linalg.solve(V, w)


def _act_fit(f, w, n_starts=6000, seed=0):
    """Exact 4-point fit w[a] = alpha*f(beta*a+gamma)+delta via random-start
    Gauss-Newton (numpy only).  Returns (beta, gamma, alpha, delta) or None."""
    w = np.asarray(w, dtype=np.float64).reshape(4)
    a4 = np.arange(4.0)
    scale = max(np.max(np.abs(w)), 1e-30)
    rng = np.random.default_rng(seed)
    best = None
    for _ in range(n_starts):
        b = rng.uniform(-4.0, 4.0)
        g = rng.uniform(-8.0, 8.0)
        M = np.stack([f(b * a4 + g), np.ones(4)], axis=1)
        sol, *_ = np.linalg.lstsq(M, w, rcond=None)
        r = M @ sol - w
        v = float(r @ r)
        if best is None or v < best[0]:
            best = (v, b, g, float(sol[0]), float(sol[1]))
    p = np.array(best[1:], dtype=np.float64)
    eps = 1e-6
    for _ in range(200):
        b, g, al, de = p
        r = al * f(b * a4 + g) + de - w
        if np.abs(r).max() < 1e-12 * scale:
            break
        J = np.empty((4, 4))
        for j in range(4):
            q = p.copy()
            q[j] += eps
            J[:, j] = (q[2] * f(q[0] * a4 + q[1]) + q[3] - w - r) / eps
        try:
            step, *_ = np.linalg.lstsq(J, r, rcond=None)
        except np.linalg.LinAlgError:
            return None
        p = p - step
    b, g, al, de = p
    r = al * f(b * a4 + g) + de - w
    if np.abs(r).max() < 1e-9 * scale and abs(al) > 1e-9 * scale:
        return float(b), float(g), float(al), float(de)
    return None


def _fit_chain(w_edge):
    """Pick the device chain for e = w_edge[a], a in {0..3}.

    Preferred: exact silu fit  e = alpha*silu(beta*a+gamma) + delta
    (one ScalarE activation + one DVE stt), then sin (domain checked).
    Falls back to the factored cubic (one ScalarE Square + 2 DVE stt).

    Returns (mode, params, lead): device computes z = chain(a) * adj_u8
    such that true adj_w = (lead/255) * z; lead/255 folds into ys.
    """
    w = np.asarray(w_edge, dtype=np.float64).reshape(4)
    v0, v1, v2, v3 = w
    scale = max(np.max(np.abs(w)), 1e-30)

    def silu(x):
        return x / (1.0 + np.exp(-np.clip(x, -60, 60)))

    fit = _act_fit(silu, w)
    if fit is not None and abs(fit[0]) * 3 + abs(fit[1]) < 30.0:
        b, g, al, de = fit
        return "silu", dict(beta=b, gamma=g, k=float(de / al)), al

    den = (v0 + v2) + 2.0 * v2 - (v1 + v3) - 2.0 * v1
    if abs(den) > 1e-9 * scale:
        d = ((v0 + v2) * v2 - (v1 + v3) * v1) / den
        if abs(v1 - d) > 1e-9 * scale:
            c = (v0 + v2 - 2.0 * d) / (2.0 * (v1 - d))
            if abs(c) < 1.0 - 1e-7:
                b = float(np.arccos(c))
                sb = np.sin(b)
                Pv = v0 - d
                Qv = ((v1 - d) - Pv * c) / sb
                alpha = float(np.hypot(Pv, Qv))
                g = float(np.arctan2(Pv, Qv))
                args = b * np.arange(4.0) + g
                fitv = alpha * np.sin(args) + d
                if (
                    np.abs(fitv - w).max() < 1e-9 * scale
                    and alpha > 1e-9 * scale
                    and np.abs(args).max() <= np.pi
                ):
                    return ("sin", dict(beta=b, gamma=g, k=float(d / alpha)), alpha)

    c0, c1, c2, c3 = _poly_coeffs(w)
    tol = 1e-7 * scale
    if abs(c3) > tol:
        A, Bc, Cc = c2 / c3, c1 / c3, c0 / c3
        roots = np.roots([1.0, A, Bc, Cc])
        r = float(np.real(roots[np.argmin(np.abs(np.imag(roots)))]))
        p = A + r
        q = Bc + p * r
        return "cubic", dict(r=r, h=p / 2.0, v2=q - p * p / 4.0), c3
    if abs(c2) > tol:
        p2, q2 = c1 / c2, c0 / c2
        return "quad", dict(h=p2 / 2.0, v2=q2 - p2 * p2 / 4.0), c2
    if abs(c1) > tol:
        return "linear", dict(r=-c0 / c1), c1
    return "const", dict(), c0


class _Emitter:
    """Holds build state; emits device ops in explicit global order."""

    def __init__(self, nc, pools, dram, mode, params):
        from concourse import mybir

        self.nc = nc
        self.pools = pools
        self.dram = dram
        self.mode = mode
        self.params = params
        self.OP = mybir.AluOpType
        self.AF = mybir.ActivationFunctionType
        self.blob = {}  # (g, ch) -> blob tile
        self.ys = {}
        self.rtp = {}
        self.ident = {}
        self.pk = {}  # g -> packed psum accumulator [128, 512]
        self.chunk_off = np.cumsum([0] + [c[0] for c in CHUNKS[:-1]])
        self.n_tiles = sum(c[0] for c in CHUNKS)

    # ---- DMA issues ------------------------------------------------------
    def issue_blob(self, g, ch):
        nc, pools = self.nc, self.pools
        w = CHUNKS[ch][0] * _N
        cb = 2 * w + (2304 if ch == XCARRIER else 0)
        off = 2 * self.chunk_off[ch] * _N + (2304 if ch > XCARRIER else 0)
        t = pools["blobp"].tile(
            [_P, cb], pools["u8"], name=f"blob{g}_{ch}", tag=f"blob{ch}", bufs=2
        )
        if ch == 0 and SPLIT_C0 and g == 0:
            # column-split halves land via BOTH rings in parallel
            hb = cb // 2
            nc.sync.dma_start(
                out=t[:, 0:hb], in_=self.dram["blob"][g, :, off : off + hb]
            )
            nc.scalar.dma_start(
                out=t[:, hb:cb], in_=self.dram["blob"][g, :, off + hb : off + cb]
            )
        elif STRIPE:
            # partition-halved across BOTH HWDGE rings: 2x arrival speed,
            # queues stay byte-balanced with no ring assignment tuning.
            nc.sync.dma_start(
                out=t[0:64, :], in_=self.dram["blob"][g, 0:64, off : off + cb]
            )
            nc.scalar.dma_start(
                out=t[64:128, :], in_=self.dram["blob"][g, 64:128, off : off + cb]
            )
        else:
            eng = {"sync": nc.sync, "scalar": nc.scalar, "pool": nc.gpsimd}[
                RING[(g, ch)]
            ]
            eng.dma_start(out=t[:], in_=self.dram["blob"][g, :, off : off + cb])
        self.blob[(g, ch)] = t
        if ch == XCARRIER:
            self.ys[g] = t[:, 2 * w : 2 * w + 1024].bitcast(pools["bf16"])
            self.rtp[g] = t[:, 2 * w + 1024 : 2 * w + 2048].bitcast(pools["f16"])
            self.ident[g] = t[:, 2 * w + 2048 : 2 * w + 2304].bitcast(pools["f16"])

    # ---- compute ---------------------------------------------------------
    def _decode_slice(self, g, ch, adj_a, ea_a, w, name):
        """silu + (s+k)*adj producing a z tile for one decode slice."""
        nc, pools, OP, AF = self.nc, self.pools, self.OP, self.AF
        z = pools["awp"].tile([_P, w], pools["bf16"], name=f"z{name}", tag="z")
        if self.mode in ("silu", "sin"):
            s_t = pools["sp"].tile([_P, w], pools["f16"], name=f"s{name}", tag="s")
            nc.scalar.activation(
                s_t[:], ea_a,
                AF.Silu if self.mode == "silu" else AF.Sin,
                bias=pools["abias_sb"][:, 0:1],
                scale=float(self.params["beta"]),
            )
            if (g, ch) in POOL_MULT:
                nc.vector.tensor_scalar(
                    s_t[:], s_t[:], float(self.params["k"]), None, OP.add
                )
                nc.gpsimd.tensor_tensor(z[:], s_t[:], adj_a, OP.mult)
            else:
                nc.vector.scalar_tensor_tensor(
                    z[:], s_t[:], float(self.params["k"]), adj_a, OP.add, OP.mult
                )
        elif self.mode in ("cubic", "quad"):
            s_t = pools["sp"].tile([_P, w], pools["f16"], name=f"s{name}", tag="s")
            nc.scalar.activation(
                s_t[:], ea_a, AF.Square,
                bias=pools["abias_sb"][:, 0:1], scale=1.0,
            )
            if self.mode == "cubic":
                qt = pools["qtp"].tile([_P, w], pools["f16"], name=f"q{name}", tag="q")
                nc.vector.scalar_tensor_tensor(
                    qt[:], ea_a, float(self.params["r"]), adj_a,
                    OP.subtract, OP.mult,
                )
                nc.vector.scalar_tensor_tensor(
                    z[:], s_t[:], float(self.params["v2"]), qt[:],
                    OP.add, OP.mult,
                )
            else:
                nc.vector.scalar_tensor_tensor(
                    z[:], s_t[:], float(self.params["v2"]), adj_a,
                    OP.add, OP.mult,
                )
        elif self.mode == "linear":
            nc.vector.scalar_tensor_tensor(
                z[:], ea_a, float(self.params["r"]), adj_a,
                OP.subtract, OP.mult,
            )
        else:  # const
            nc.vector.tensor_copy(z[:], adj_a)
        return z

    def seed(self, g):
        """Open the graph's single PSUM group with outT = I.T @ rtp."""
        self.pk[g] = self.pools["ps"].tile(
            [_P, _H], self.pools["f32"], tag=f"pk{g}", name=f"pk{g}"
        )
        self.nc.tensor.matmul(
            self.pk[g][:],
            lhsT=self.ident[g][:, :],
            rhs=self.rtp[g][:, :],
            start=True, stop=False,
            skip_group_check=True,
        )

    def chunk_ops(self, g, ch):
        """Decode slices of DMA chunk ch, each followed by its agg matmuls."""
        nc = self.nc
        wT = CHUNKS[ch][0] * _N
        blob_t = self.blob[(g, ch)]
        if ch == 0 and SPLIT_C0:
            # col-split layout [adjA|eaA|adjB|eaB]: slice h feeds node-half h
            for h in range(2):
                adj_a = blob_t[:, h * _N : h * _N + _H]
                ea_a = blob_t[:, h * _N + _H : (h + 1) * _N]
                z = self._decode_slice(g, ch, adj_a, ea_a, _H, f"{g}_{ch}_{h}")
                nc.tensor.matmul(
                    self.pk[g][h * _C : (h + 1) * _C, :],
                    lhsT=self.ys[g][:, 0:_C],
                    rhs=z[:, :],
                    start=False, stop=False,
                    skip_group_check=True,
                )
            return
        s_off = 0  # tile offset within the DMA chunk
        for si, stiles in enumerate(CHUNKS[ch][1]):
            w = stiles * _N
            adj_a = blob_t[:, s_off * _N : s_off * _N + w]
            ea_a = blob_t[:, wT + s_off * _N : wT + s_off * _N + w]
            jt0 = self.chunk_off[ch] + s_off
            if (
                SPLIT_TAIL
                and stiles == 1
                and jt0 == self.n_tiles - 1
                and self.mode in ("silu", "sin")
            ):
                # final slice: silu full-width, stt per column half so the
                # h0 matmul overlaps the h1 stt
                OP, AF, pools = self.OP, self.AF, self.pools
                s_t = pools["sp"].tile(
                    [_P, w], pools["f16"], name=f"s{g}_{ch}_{si}", tag="s"
                )
                nc.scalar.activation(
                    s_t[:], ea_a,
                    AF.Silu if self.mode == "silu" else AF.Sin,
                    bias=pools["abias_sb"][:, 0:1],
                    scale=float(self.params["beta"]),
                )
                z = pools["awp"].tile(
                    [_P, w], pools["bf16"], name=f"z{g}_{ch}_{si}", tag="z"
                )
                for h in range(2):
                    sl = slice(h * _H, (h + 1) * _H)
                    nc.vector.scalar_tensor_tensor(
                        z[:, sl], s_t[:, sl], float(self.params["k"]),
                        adj_a[:, sl], OP.add, OP.mult,
                    )
                    nc.tensor.matmul(
                        self.pk[g][h * _C : (h + 1) * _C, :],
                        lhsT=self.ys[g][:, jt0 * _C : (jt0 + 1) * _C],
                        rhs=z[:, sl],
                        start=False, stop=(h == 1),
                        skip_group_check=True,
                    )
                s_off += stiles
                continue
            z = self._decode_slice(g, ch, adj_a, ea_a, w, f"{g}_{ch}_{si}")
            for sub in range(stiles):
                jt = self.chunk_off[ch] + s_off + sub
                last = jt == self.n_tiles - 1
                for half in range(2):
                    nc.tensor.matmul(
                        self.pk[g][half * _C : (half + 1) * _C, :],
                        lhsT=self.ys[g][:, jt * _C : (jt + 1) * _C],
                        rhs=z[:, sub * _N + half * _H : sub * _N + (half + 1) * _H],
                        start=False, stop=(last and half == 1),
                        skip_group_check=True,
                    )
            s_off += stiles

    def tail_out(self, g):
        """PSUM -> f16 copy split by partition halves (each waits only its
        own half's last matmul; vector+scalar run concurrently), then two
        stores on the sync ring."""
        outb = self.pools["outp"].tile([_P, _H], self.pools["f16"], tag=f"outb{g}")
        self.nc.vector.tensor_copy(outb[0:_C, :], self.pk[g][0:_C, :])
        self.nc.scalar.copy(out=outb[_C:_P, :], in_=self.pk[g][_C:_P, :])
        self.nc.sync.dma_start(out=self.dram["out"][g, 0:_C, :], in_=outb[0:_C, :])
        self.nc.sync.dma_start(out=self.dram["out"][g, _C:_P, :], in_=outb[_C:_P, :])


def _build_module(mode, params):
    import concourse.bass as bass  # noqa: F401
    from concourse import bacc, mybir
    from concourse.tile import TileContext

    f32 = mybir.dt.float32
    f16 = mybir.dt.float16
    bf16 = mybir.dt.bfloat16
    u8 = mybir.dt.uint8

    nc = bacc.Bacc(
        "TRN2", target_bir_lowering=False, debug=False, num_devices=_NCORES
    )

    blob_w = _NT * _N * 2 + 2304
    dram = {
        "blob": nc.dram_tensor("blob", [_G, _P, blob_w], u8, kind="ExternalInput"),
        "out": nc.dram_tensor("out", [_G, _P, _H], f16, kind="ExternalOutput"),
    }

    pool_specs = [
        ("consts", 1, None),
        ("blobp", 1, None),
        ("sp", SP_BUFS, None),
        ("awp", AW_BUFS, None),
        ("outp", 1, None),
        ("ps", 1, "PSUM"),
    ]
    if mode == "cubic":
        pool_specs.insert(3, ("qtp", 2, None))

    with TileContext(nc) as tc, ExitStack() as ctx:
        pools = {"f32": f32, "f16": f16, "bf16": bf16, "u8": u8, "tc": tc}
        for name, bufs, space in pool_specs:
            kw = {"space": space} if space else {}
            pools[name] = ctx.enter_context(tc.tile_pool(name=name, bufs=bufs, **kw))

        prio = tc.high_priority() if PRIO_CONSTS else None
        if prio:
            prio.__enter__()
        if mode in ("silu", "sin", "cubic", "quad"):
            ab = pools["consts"].tile([_P, 1], f32, tag="abias")
            bias_val = params["gamma"] if mode in ("silu", "sin") else params["h"]
            nc.vector.memset(ab[:], float(bias_val))
            pools["abias_sb"] = ab

        if mode in ("silu", "sin"):
            # dummy activation exactly mirroring the real one (u8 input,
            # f16 out, AP bias + float scale) as the FIRST Scalar
            # instruction: pulls the ACT_TABLE_LOAD into startup dead-time.
            dmu = pools["consts"].tile([_P, 1], u8, tag="dummy_in")
            nc.gpsimd.memset(dmu[:], 0)
            dout = pools["consts"].tile([_P, 1], f16, tag="dummy_out")
            af = (
                mybir.ActivationFunctionType.Silu
                if mode == "silu"
                else mybir.ActivationFunctionType.Sin
            )
            nc.scalar.activation(
                dout[:], dmu[:], af, bias=ab[:, 0:1],
                scale=float(params["beta"]),
            )
        if prio:
            prio.__exit__(None, None, None)

        em = _Emitter(nc, pools, dram, mode, params)

        # ---- all input DMA issues in execution order at priority 0 so the
        # 8 HWDGE semaphore lanes recycle without false serialization.
        with tc.high_priority():
            for item in ISSUE_ORDER:
                em.issue_blob(*item)

        for g in range(_G):
            em.seed(g)
            for ch in range(len(CHUNKS)):
                em.chunk_ops(g, ch)
            em.tail_out(g)

    nc.finalize()
    return nc


def _get_module(w_edge):
    mode, params, lead = _fit_chain(w_edge)
    key = (
        mode,
        tuple(sorted((k, round(v, 15)) for k, v in params.items())),
        tuple((t, tuple(s)) for t, s in CHUNKS),
        tuple(sorted(RING.items())),
        tuple(POOL_MULT),
        tuple(map(str, ISSUE_ORDER)),
        STRIPE,
        PRIO_CONSTS,
        SP_BUFS,
        AW_BUFS,
        SPLIT_C0,
        XCARRIER,
        SPLIT_TAIL,
    )
    if key not in _BUILD_CACHE:
        _BUILD_CACHE[key] = _build_module(mode, params)
    return _BUILD_CACHE[key], lead


def kernel(x, adj, edge_attr, W_rel, b_rel, W_root, w_edge):
    global LAST_RESULTS
    from concourse import mybir
    from concourse.bass_utils import run_bass_kernel_spmd

    bf16np = mybir.dt.np(mybir.dt.bfloat16)
    f16np = mybir.dt.np(mybir.dt.float16)

    x = np.asarray(x, dtype=np.float32)
    adj = np.asarray(adj, dtype=np.float32)
    ea = np.asarray(edge_attr, dtype=np.int32).reshape(_B, _N, _N)
    W_rel = np.asarray(W_rel, dtype=np.float64)
    W_root = np.asarray(W_root, dtype=np.float64)
    b_rel = np.asarray(b_rel, dtype=np.float64).reshape(1, _C)
    w_edge = np.asarray(w_edge)

    nc, lead = _get_module(w_edge)

    def tile_rows(a):
        """[B, N, F] -> [B, 128, NT*F]: row j*128+p of graph b lands at
        [b, p, j*F:(j+1)*F] -- one contiguous free-dim line per partition."""
        B, N, F = a.shape
        return np.ascontiguousarray(
            a.reshape(B, _NT, _P, F).transpose(0, 2, 1, 3).reshape(B, _P, _NT * F)
        )

    # transposed + row-tiled layouts; adj quantized to u8 (x255)
    adjq = np.rint(
        tile_rows(np.ascontiguousarray(adj.transpose(0, 2, 1))) * 255.0
    ).astype(np.uint8)
    eaT = tile_rows(np.ascontiguousarray(ea.transpose(0, 2, 1))).astype(np.uint8)

    # associativity folds: ys = x @ (lead/255 * W_rel); rt = x @ W_root + b_rel
    x64 = x.astype(np.float64)
    ys = tile_rows((x64 @ (lead / 255.0 * W_rel)).astype(np.float32)).astype(bf16np)
    rt = (x64 @ W_root + b_rel).astype(np.float32)  # [B, N, C]
    # rtp[g, h*64+c, i] = rt[g, h*512+i, c]
    rtp = np.ascontiguousarray(
        rt.reshape(_B, 2, _H, _C).transpose(0, 1, 3, 2).reshape(_B, _P, _H)
    ).astype(f16np)
    xpack = np.ascontiguousarray(
        np.concatenate([ys.view(np.uint8), rtp.view(np.uint8)], axis=2)
    )
    ident = np.eye(_P, dtype=f16np)
    blob = _pack_blob(adjq, eaT, xpack, ident)

    in_maps = []
    for c in range(_NCORES):
        sl = slice(c * _G, (c + 1) * _G)
        in_maps.append({"blob": blob[sl]})

    res = run_bass_kernel_spmd(nc, in_maps, list(range(_NCORES)), trace=TRACE)
    LAST_RESULTS = res
    # out staged [G, 128, 512] f16: out[g, h*512+i, c] = outs[g, h*64+c, i]
    outs = np.concatenate(
        [np.asarray(res.results[c]["out"]) for c in range(_NCORES)], axis=0
    ).astype(np.float32)
    out = (
        outs.reshape(_B, 2, _C, _H)
        .transpose(0, 1, 3, 2)
        .reshape(_B, _N, _C)
    )
    return np.ascontiguousarray(out)
